# revision 1
# baseline (speedup 1.0000x reference)
"""BiLSTM (2-layer, bidirectional) encoder + attention pooling on 8 Trainium2 cores.

Topology (one SPMD program, roles selected by partition id):
  pid 0: layer-0 forward LSTM  (full batch 128)          role 0
  pid 2: layer-0 backward LSTM (host-reversed x)         role 0
  pid 1: layer-1 forward LSTM + f-side attention + f-half output   role 1
  pid 3: layer-1 backward LSTM + b-side attention + b-half output  role 1
  pid 4-7: idle (participate in collectives with garbage pairs)

Communication per 32-step chunk (all bf16):
  AG1 groups [[0,1],[2,3],[4,5],[6,7]]: L0 hidden states (transposed layout)
      2MB wire -> L1 consumes with lag 2.
  AG2 groups [[1,3],[0,2],[4,6],[5,7]]: A=64 attention pre-projections
      0.5MB wire -> peer attention scores.

Gates are reordered host-side to [g|o|i|f] so bank0 of the PSUM gate tile
finishes first and tanh(g) can start early; tanh and sigmoid share one ACT
table set so there are no table reloads.  exp for the softmax uses the exact
identity exp(z) = sig(z)/(1-sig(z)) with a fixed shift M = sum|Wa2| so scores
can be consumed in any order; windows run middle-out as both directions'
scores become available.
"""
import numpy as np

B, D, H, A = 128, 256, 256, 64
CH = 32

_BUILD_CACHE = {}
_last_in_maps = None


def _build(S):
    import concourse.bass as bass
    from concourse import bacc
    import concourse.mybir as mybir
    from concourse.tile import TileContext
    from concourse.masks import make_identity

    F32 = mybir.dt.float32
    BF16 = mybir.dt.bfloat16
    AF = mybir.ActivationFunctionType
    OP = mybir.AluOpType

    NCH = S // CH
    LAG = 2                       # L1 consumes L0 chunk j at tick j+LAG
    NW = NCH // 2                 # middle-out window pairs
    WSTART = NCH // 2 + LAG + 2   # first window tick
    NT = WSTART + NW + 1          # + finalize tick
    MID = S // 2

    G1 = [[0, 1], [2, 3], [4, 5], [6, 7]]
    G2 = [[1, 3], [0, 2], [4, 6], [5, 7]]

    nc = bacc.Bacc("TRN2", target_bir_lowering=False, debug=False, num_devices=8)

    xs_d = nc.dram_tensor("xs", [S, 2, 128, B], BF16, kind="ExternalInput")
    wih_d = nc.dram_tensor("wih", [128, 2, 2, 512], BF16, kind="ExternalInput")
    whh_d = nc.dram_tensor("whh", [128, 2, 2, 512], BF16, kind="ExternalInput")
    bias_d = nc.dram_tensor("bias", [1, 1024], BF16, kind="ExternalInput")
    wa1_d = nc.dram_tensor("wa1t", [128, 2, A], BF16, kind="ExternalInput")
    wa2_d = nc.dram_tensor("wa2r", [1, A], BF16, kind="ExternalInput")
    mneg_d = nc.dram_tensor("mneg", [1, 1], F32, kind="ExternalInput")
    yout_d = nc.dram_tensor("yout", [B, H], F32, kind="ExternalOutput")

    agin1 = [nc.dram_tensor(f"agin1_{j}", [CH, 128, 256], BF16) for j in range(NCH)]
    agbuf1 = [nc.dram_tensor(f"agbuf1_{j}", [2 * CH, 128, 256], BF16)
              for j in range(NCH)]
    agin2 = [nc.dram_tensor(f"agin2_{j}", [128, CH, A], BF16) for j in range(NCH)]
    agbuf2 = [nc.dram_tensor(f"agbuf2_{j}", [2 * 128, CH, A], BF16)
              for j in range(NCH)]
    hist_d = nc.dram_tensor("hist", [NCH, 128, CH, 256], BF16)
    # per-pid slot pull lands here (ring of 2 window ticks, 2 blocks each)
    aremd = [nc.dram_tensor(f"aremd_{k}", [2, 128, CH, A], BF16) for k in range(2)]

    with TileContext(nc) as tc:
        with tc.tile_pool(name="wpool", bufs=1) as wp, \
             tc.tile_pool(name="xpool", bufs=2) as xp, \
             tc.tile_pool(name="gpool", bufs=2) as gp, \
             tc.tile_pool(name="apool", bufs=2) as ap_, \
             tc.tile_pool(name="psum", bufs=2, space="PSUM") as pp:

            pid = nc.partition_id()
            role = pid % 2 + (pid // 4) * 2

            # ---------------- prologue: constants and state ----------------
            wih = wp.tile([128, 2, 2, 512], BF16, tag="wih")
            nc.gpsimd.dma_start(out=wih[:], in_=wih_d.ap())
            whh = wp.tile([128, 2, 2, 512], BF16, tag="whh")
            nc.gpsimd.dma_start(out=whh[:], in_=whh_d.ap())
            bias_r = wp.tile([1, 1024], BF16, tag="bias_r")
            nc.gpsimd.dma_start(out=bias_r[:], in_=bias_d.ap())
            wa1 = wp.tile([128, 2, A], BF16, tag="wa1")
            nc.gpsimd.dma_start(out=wa1[:], in_=wa1_d.ap())
            wa2bc = wp.tile([128, A], BF16, tag="wa2bc")
            nc.sync.dma_start(out=wa2bc[:], in_=wa2_d.ap().partition_broadcast(128))
            wa2rep = wp.tile([128, CH, A], BF16, tag="wa2rep")
            for r in range(CH):
                nc.vector.tensor_copy(out=wa2rep[:, r], in_=wa2bc[:])
            mneg = wp.tile([128, 1], F32, tag="mneg")
            nc.sync.dma_start(out=mneg[:], in_=mneg_d.ap().partition_broadcast(128))
            ones_r = wp.tile([1, 128], BF16, tag="ones_r")
            nc.gpsimd.memset(ones_r[:], 1.0)
            ident = wp.tile([128, 128], F32, tag="ident")
            make_identity(nc, ident[:])
            identb = wp.tile([128, 128], BF16, tag="identb")
            nc.vector.tensor_copy(out=identb[:], in_=ident[:])

            zini = wp.tile([128, 256], F32, tag="zini")
            nc.gpsimd.memset(zini[:], 0.0)
            # persistent prev-step transposed h (per role; only one role runs
            # per core but tiles are statically allocated for both)
            hTp0 = wp.tile([128, 2, 128], BF16, tag="hTp0")
            nc.gpsimd.memset(hTp0[:], 0.0)
            hTp1 = wp.tile([128, 2, 128], BF16, tag="hTp1")
            nc.gpsimd.memset(hTp1[:], 0.0)
            hT_prev = {0: hTp0, 1: hTp1}
            c_s = wp.tile([128, 256], F32, tag="c_s")
            nc.vector.tensor_copy(out=c_s[:], in_=zini[:])

            # ping-pong input-chunk tiles (persistent; prefetched a tick ahead)
            xtA0 = wp.tile([128, CH, 2, 128], BF16, tag="xtA0")
            xtB0 = wp.tile([128, CH, 2, 128], BF16, tag="xtB0")
            xtA1 = wp.tile([128, CH, 2, 128], BF16, tag="xtA1")
            xtB1 = wp.tile([128, CH, 2, 128], BF16, tag="xtB1")
            xt_pp = {0: (xtA0, xtB0), 1: (xtA1, xtB1)}
            # role0's chunk-0 input is ready in DRAM at start: prefetch now
            nc.sync.dma_start(
                out=xtA0[:],
                in_=xs_d.ap()[0:CH].rearrange("c a p b -> p c a b"))

            def emit_xt_prefetch(cj, is_l1):
                xt = xt_pp[1 if is_l1 else 0][cj % 2]
                if is_l1:
                    nc.sync.dma_start(
                        out=xt[:],
                        in_=agbuf1[cj].ap()[0:CH]
                        .rearrange("c p (a b) -> p c a b", a=2))
                else:
                    nc.sync.dma_start(
                        out=xt[:],
                        in_=xs_d.ap()[cj * CH:(cj + 1) * CH]
                        .rearrange("c a p b -> p c a b"))

            # attention state
            acc_v = wp.tile([128, 256], F32, tag="acc_v")
            nc.vector.tensor_copy(out=acc_v[:], in_=zini[:])
            den = wp.tile([128, 1], F32, tag="den")
            nc.vector.tensor_copy(out=den[:], in_=zini[:, 0:1])

            def emit_lstm_chunk(cj, is_l1, sfx):
                """One CH-step chunk of LSTM; role0 ships hT, role1 also
                computes attention projections and h history.  Gate layout
                [2g|o|i|f]; all nonlinearities are Sigmoid (tanh z =
                2*sig(2z)-1 with the 2z folded into weights host-side)."""
                rk = 1 if is_l1 else 0
                xt = xt_pp[rk][cj % 2]
                if cj + 1 < NCH:
                    emit_xt_prefetch(cj + 1, is_l1)
                hTc = xp.tile([128, CH, 2, 128], BF16, tag="hTc", name=f"hTc{sfx}")
                if is_l1:
                    hc = xp.tile([128, CH, 256], BF16, tag="hc", name=f"hc{sfx}")
                    a_ch = ap_.tile([128, CH, A], BF16, bufs=1, tag="ach", name=f"ach{sfx}")

                pgb = [None, None]  # rotating python refs for gb tiles

                def emit_partA(t0):
                    gb = pp.tile([128, 1088], F32, tag="gb", name=f"gb{sfx}_{t0}")
                    # alternate banks so PSUM drains overlap the next fill;
                    # pair matmuls per stationary to halve weight loads
                    nc.tensor.matmul(gb[:, 0:512], ones_r[:], bias_r[:, 0:512],
                                     start=True, stop=False)
                    nc.tensor.matmul(gb[:, 512:1024], ones_r[:], bias_r[:, 512:1024],
                                     start=True, stop=False)
                    for kc in range(2):
                        nc.tensor.matmul(gb[:, 0:512], xt[:, t0, kc], wih[:, kc, 0],
                                         start=False, stop=False)
                        nc.tensor.matmul(gb[:, 512:1024], xt[:, t0, kc],
                                         wih[:, kc, 1], start=False, stop=False)
                    return gb

                def emit_partB(gb, t0):
                    if t0 == 0:
                        prevT = hT_prev[rk][:]
                    else:
                        prevT = hTc[:, t0 - 1]
                    do_aps = is_l1 and t0 >= 1
                    nc.tensor.matmul(gb[:, 0:512], prevT[:, 0], whh[:, 0, 0],
                                     start=False, stop=False)
                    nc.tensor.matmul(gb[:, 512:1024], prevT[:, 0], whh[:, 0, 1],
                                     start=False, stop=False)
                    if do_aps:
                        # attention projection of step t0-1 (same stationaries)
                        nc.tensor.matmul(gb[:, 1024:1088], prevT[:, 0], wa1[:, 0],
                                         start=True, stop=False)
                    nc.tensor.matmul(gb[:, 0:512], prevT[:, 1], whh[:, 1, 0],
                                     start=False, stop=True)
                    nc.tensor.matmul(gb[:, 512:1024], prevT[:, 1], whh[:, 1, 1],
                                     start=False, stop=True)
                    if do_aps:
                        nc.tensor.matmul(gb[:, 1024:1088], prevT[:, 1], wa1[:, 1],
                                         start=False, stop=True)
                    sg = gp.tile([128, 1024], F32, tag="sg", name=f"sg{sfx}_{t0}")
                    nc.scalar.activation(sg[:, 0:512], gb[:, 0:512], AF.Sigmoid)
                    nc.scalar.activation(sg[:, 512:1024], gb[:, 512:1024],
                                         AF.Sigmoid)
                    # layout: sig_2g | sig_o | sig_i | sig_f
                    tg = gp.tile([128, 256], F32, tag="tg", name=f"tg{sfx}_{t0}")
                    nc.gpsimd.tensor_scalar(out=tg[:], in0=sg[:, 0:256],
                                            scalar1=2.0, scalar2=-1.0,
                                            op0=OP.mult, op1=OP.add)
                    v = gp.tile([128, 256], F32, tag="v", name=f"v{sfx}_{t0}")
                    nc.gpsimd.tensor_mul(out=v[:], in0=tg[:], in1=sg[:, 512:768])
                    nc.vector.tensor_mul(out=c_s[:], in0=c_s[:], in1=sg[:, 768:1024])
                    nc.vector.tensor_add(out=c_s[:], in0=c_s[:], in1=v[:])
                    th = gp.tile([128, 256], F32, tag="th", name=f"th{sfx}_{t0}")
                    nc.scalar.activation(th[:], c_s[:], AF.Sigmoid, scale=2.0)
                    w_t = gp.tile([128, 256], F32, tag="w_t", name=f"w{sfx}_{t0}")
                    nc.vector.tensor_scalar(out=w_t[:], in0=th[:],
                                            scalar1=2.0, scalar2=-1.0,
                                            op0=OP.mult, op1=OP.add)
                    if is_l1:
                        hdst = hc[:, t0]
                    else:
                        h0 = gp.tile([128, 256], BF16, tag="h0", name=f"h0{sfx}_{t0}")
                        hdst = h0[:]
                    nc.vector.tensor_mul(out=hdst, in0=sg[:, 256:512], in1=w_t[:])
                    tp = pp.tile([128, 256], BF16, tag="tp", name=f"tp{sfx}_{t0}")
                    nc.tensor.transpose(tp[:, 0:128], hdst[:, 0:128], identb[:])
                    nc.tensor.transpose(tp[:, 128:256], hdst[:, 128:256], identb[:])
                    nc.vector.tensor_copy(
                        out=hTc[:, t0].rearrange("p a b -> p (a b)"), in_=tp[:])
                    if is_l1 and t0 >= 1:
                        nc.vector.tensor_copy(out=a_ch[:, t0 - 1],
                                              in_=gb[:, 1024:1088])

                pgb[0] = emit_partA(0)
                for t0 in range(CH):
                    if t0 + 1 < CH:
                        pgb[1] = emit_partA(t0 + 1)
                    emit_partB(pgb[0], t0)
                    pgb[0] = pgb[1]

                if is_l1:
                    # tail: attention projection of step CH-1
                    aps_t = pp.tile([128, 64], F32, tag="tp", name=f"apst{sfx}")
                    nc.tensor.matmul(aps_t[:], hTc[:, CH - 1, 0], wa1[:, 0],
                                     start=True, stop=False)
                    nc.tensor.matmul(aps_t[:], hTc[:, CH - 1, 1], wa1[:, 1],
                                     start=False, stop=True)
                    nc.vector.tensor_copy(out=a_ch[:, CH - 1], in_=aps_t[:])
                    nc.sync.dma_start(out=agin2[cj].ap(), in_=a_ch[:])
                    nc.sync.dma_start(out=hist_d.ap()[cj], in_=hc[:])
                else:
                    nc.sync.dma_start(
                        out=agin1[cj].ap().rearrange("c p f -> p c f"),
                        in_=hTc[:].rearrange("p c a b -> p c (a b)"))
                nc.vector.tensor_copy(
                    out=hT_prev[1 if is_l1 else 0][:].rearrange("p a b -> p (a b)"),
                    in_=hTc[:, CH - 1].rearrange("p a b -> p (a b)"))

            def emit_window(w, sfx):
                """Score + weighted-accumulate for the window-pair w
                (own-time blocks around MID). Slot-independent part."""
                for bi, bs in enumerate((MID - CH * (w + 1), MID + CH * w)):
                    cf = bs // CH
                    aown = ap_.tile([128, CH, A], BF16, bufs=1, tag="aown",
                                    name=f"aown{sfx}_{bi}")
                    nc.sync.dma_start(out=aown[:], in_=agin2[cf].ap())
                    arem = ap_.tile([128, CH, A], BF16, bufs=1, tag="arem",
                                    name=f"arem{sfx}_{bi}")
                    nc.sync.dma_start(out=arem[:], in_=aremd[w % 2].ap()[bi])
                    asum = ap_.tile([128, CH, A], BF16, bufs=1, tag="asum",
                                    name=f"asum{sfx}_{bi}")
                    nc.vector.tensor_add(out=asum[:], in0=aown[:],
                                         in1=arem[:][:, ::-1, :])
                    # tanh via sigmoid: 2*sig(2z)-1; the 2z is folded into wa1,
                    # the 2* into wa2rep, and the -1 into the mneg shift
                    nc.scalar.activation(asum[:], asum[:], AF.Sigmoid)
                    nc.vector.tensor_mul(out=asum[:], in0=asum[:], in1=wa2rep[:])
                    sco = ap_.tile([128, CH], F32, tag="sco", name=f"sco{sfx}_{bi}")
                    nc.vector.tensor_reduce(out=sco[:], in_=asum[:],
                                            axis=mybir.AxisListType.X,
                                            op=OP.add)
                    nc.scalar.activation(sco[:], sco[:], AF.Sigmoid,
                                         bias=mneg[:, 0:1])
                    dtl = ap_.tile([128, CH], F32, tag="dtl", name=f"dtl{sfx}_{bi}")
                    nc.vector.tensor_scalar(out=dtl[:], in0=sco[:],
                                            scalar1=-1.0, scalar2=1.0,
                                            op0=OP.mult, op1=OP.add)
                    nc.vector.reciprocal(out=dtl[:], in_=dtl[:])
                    e_blk = ap_.tile([128, CH], F32, tag="e_blk",
                                     name=f"eb{sfx}_{bi}")
                    dinc = ap_.tile([128, 1], F32, tag="dinc",
                                    name=f"di{sfx}_{bi}")
                    nc.vector.scalar_tensor_tensor(
                        out=e_blk[:], in0=sco[:], scalar=1.0, in1=dtl[:],
                        op0=OP.mult, op1=OP.mult, accum_out=dinc[:])
                    nc.vector.tensor_add(out=den[:], in0=den[:], in1=dinc[:])
                    hw_ = ap_.tile([128, CH, 256], BF16, bufs=1, tag="hw",
                                   name=f"hw{sfx}_{bi}")
                    nc.sync.dma_start(out=hw_[:], in_=hist_d.ap()[cf])
                    for u in range(CH):
                        nc.vector.scalar_tensor_tensor(
                            out=acc_v[:], in0=hw_[:, u], scalar=e_blk[:, u:u + 1],
                            in1=acc_v[:], op0=OP.mult, op1=OP.add)

            # ---------------- tick loop ----------------
            for tick in range(NT):
                if tick < NCH:
                    with tc.If(role == 0, name=f"L0t{tick}"):
                        emit_lstm_chunk(tick, False, f"a{tick}")

                if tick == 1:
                    # role1's chunk-0 input became available via AG1[0]
                    with tc.If(role == 1, name="L1pf"):
                        emit_xt_prefetch(0, True)

                j1 = tick - LAG
                w = tick - WSTART

                # slot-asymmetric DMA: pull the peer's a-chunk(s) for window w
                # into the aremd DRAM mirror (pid1 reads slot1, pid3 slot0)
                if 0 <= w < NW:
                    for slot, cpid in ((1, 1), (0, 3)):
                        with tc.If(pid == cpid, name=f"AR{tick}_{cpid}"):
                            for bi, bs in enumerate((MID - CH * (w + 1),
                                                     MID + CH * w)):
                                c_rem = NCH - 1 - bs // CH
                                nc.sync.dma_start(
                                    out=aremd[w % 2].ap()[bi],
                                    in_=agbuf2[c_rem].ap()
                                    [slot * 128:(slot + 1) * 128])

                do_l1 = 0 <= j1 < NCH
                do_w = 0 <= w < NW
                if do_l1 or do_w:
                    with tc.If(role == 1, name=f"L1t{tick}"):
                        if do_l1:
                            emit_lstm_chunk(j1, True, f"b{tick}")
                        if do_w:
                            emit_window(w, f"w{tick}")

                if tick == NT - 1:
                    with tc.If(role == 1, name="fin"):
                        rden = ap_.tile([128, 1], F32, tag="dinc", name="rden")
                        nc.vector.reciprocal(out=rden[:], in_=den[:])
                        yt = ap_.tile([128, 256], F32, tag="yt", name="yt")
                        nc.vector.tensor_scalar_mul(yt[:], acc_v[:], rden[:, 0:1])
                        nc.sync.dma_start(out=yout_d.ap(), in_=yt[:])

                # ---------------- collectives ----------------
                if tick < NCH:
                    nc.gpsimd.collective_compute(
                        "AllGather", mybir.AluOpType.bypass,
                        replica_groups=G1,
                        ins=[agin1[tick].ap()], outs=[agbuf1[tick].ap()])
                if 0 <= j1 < NCH:
                    nc.gpsimd.collective_compute(
                        "AllGather", mybir.AluOpType.bypass,
                        replica_groups=G2,
                        ins=[agin2[j1].ap()], outs=[agbuf2[j1].ap()])

    nc.compile()
    return nc


def _prep_lstm_w(Wih, Whh, bih, bhh, bf16):
    # torch gate order i,f,g,o -> device order 2g,o,i,f (bank0=[2g|o],
    # bank1=[i|f]); the 2x on g feeds tanh(z) = 2*sig(2z)-1
    def reorder(M):
        return np.concatenate([2.0 * M[512:768], M[768:1024], M[0:256],
                               M[256:512]], axis=0)

    wih_t = np.ascontiguousarray(reorder(np.asarray(Wih, np.float32)).T)
    whh_t = np.ascontiguousarray(reorder(np.asarray(Whh, np.float32)).T)
    bias = reorder((np.asarray(bih, np.float32)
                    + np.asarray(bhh, np.float32)).reshape(1024, 1)).reshape(1, 1024)

    def chunk(WT):
        return np.ascontiguousarray(
            WT.reshape(2, 128, 2, 512).transpose(1, 0, 2, 3)).astype(bf16)

    return chunk(wih_t), chunk(whh_t), bias.astype(bf16)


def kernel(**inputs):
    import ml_dtypes
    from concourse.bass_utils import run_bass_kernel_spmd

    bf16 = ml_dtypes.bfloat16
    x = np.asarray(inputs["x"], np.float32)
    Bv, S, Dv = x.shape
    if (S, "nc") not in _BUILD_CACHE:
        _BUILD_CACHE[(S, "nc")] = _build(S)
    nc = _BUILD_CACHE[(S, "nc")]

    xs_f = np.ascontiguousarray(x.transpose(1, 2, 0)).reshape(
        S, 2, 128, Bv).astype(bf16)
    xs_b = np.ascontiguousarray(x[:, ::-1].transpose(1, 2, 0)).reshape(
        S, 2, 128, Bv).astype(bf16)
    z_xs = np.zeros((S, 2, 128, Bv), bf16)
    zw = np.zeros((128, 2, 2, 512), bf16)
    zb = np.zeros((1, 1024), bf16)

    wf0 = _prep_lstm_w(inputs["Wih_f0"], inputs["Whh_f0"], inputs["bih_f0"],
                       inputs["bhh_f0"], bf16)
    wf1 = _prep_lstm_w(inputs["Wih_f1"], inputs["Whh_f1"], inputs["bih_f1"],
                       inputs["bhh_f1"], bf16)
    wb0 = _prep_lstm_w(inputs["Wih_b0"], inputs["Whh_b0"], inputs["bih_b0"],
                       inputs["bhh_b0"], bf16)
    wb1 = _prep_lstm_w(inputs["Wih_b1"], inputs["Whh_b1"], inputs["bih_b1"],
                       inputs["bhh_b1"], bf16)

    wa1 = 2.0 * np.asarray(inputs["Wa1"], np.float32)    # [A, 2H]; 2z of tanh
    wa2 = np.asarray(inputs["Wa2"], np.float32).reshape(1, A)

    def wa1half(cols):
        # [A, 256] -> [128, 2, A] (kc chunks of the 256 input dims)
        return np.ascontiguousarray(
            cols.T.reshape(2, 128, A).transpose(1, 0, 2)).astype(bf16)

    wa1f = wa1half(wa1[:, 0:H])
    wa1b = wa1half(wa1[:, H:2 * H])
    zwa1 = np.zeros((128, 2, A), bf16)
    # score uses sum(2*wa2 * sig(2z)) with the constant -sum(wa2) absorbed
    # into the fixed softmax shift M
    wa2b = (2.0 * wa2).astype(bf16)
    mconst = np.float32(np.maximum(2.0 * wa2, 0.0).sum())
    mneg = np.full((1, 1), -mconst, np.float32)

    def imap(xs, w3, wa1t):
        wih, whh, bias = w3
        return {"xs": xs, "wih": wih, "whh": whh, "bias": bias,
                "wa1t": wa1t, "wa2r": wa2b, "mneg": mneg}

    zero3 = (zw, zw, zb)
    in_maps = [
        imap(xs_f, wf0, zwa1), imap(z_xs, wf1, wa1f),
        imap(xs_b, wb0, zwa1), imap(z_xs, wb1, wa1b),
        imap(z_xs, zero3, zwa1), imap(z_xs, zero3, zwa1),
        imap(z_xs, zero3, zwa1), imap(z_xs, zero3, zwa1),
    ]
    global _last_in_maps
    _last_in_maps = in_maps
    res = run_bass_kernel_spmd(nc, in_maps, core_ids=list(range(8)))
    out = np.concatenate([res.results[1]["yout"], res.results[3]["yout"]], axis=1)
    return out.astype(np.float32)



# revision 16
# speedup vs baseline: 1.4200x; 1.4200x over previous
"""BiLSTM (2-layer, bidirectional) encoder + attention pooling on 8 Trainium2 cores.

Topology (one SPMD program, roles selected by partition id):
  pid 0: layer-0 forward LSTM   (full batch 128)      role 0
  pid 2: layer-0 backward LSTM  (host-reversed x)     role 0
  pid 1: layer-1 forward LSTM                         role 1
  pid 3: layer-1 backward LSTM                        role 1
  pid 5: f-side attention (a_pre, scores, pooling)    role 3
  pid 7: b-side attention                             role 3
  pid 4, 6: idle (participate in collectives with garbage pairs)

LSTM inner loop uses a transposed-gate layout: gates live in PSUM as
[4H-on-partitions x batch-free] tiles (8 gc tiles of [128,128] per step;
bank b holds gate-chunk pair (2b, 2b+1) for a 2-step group, ping-ponged).
Per group the PE prefills each bank with bias (rank-2 outer-product
matmul against a half-hot moving tile) + x@Wih (batched N=256 matmuls);
the recurrent Whh matmuls accumulate on top (start=False), so no
separate adds are needed and the PE stays dense (HAM stays warm).
h^T is produced directly by the DVE h-mul (no per-step transposes).
Gate nonlinearities use Sigmoid and true Tanh (same ACT table set).

Communication per 32-step chunk (all bf16):
  AG_h0 groups [[0,1],[2,3],..]: L0 hidden chunks -> L1 (lag 2 ticks).
  AG_h1 groups [[1,5],[3,7],..]: L1 hidden chunks -> attention helpers.
  AG_ap groups [[5,7],..]: A=64 attention pre-projections exchanged
      between the two helpers.
softmax uses exp(z) = sig(z)/(1-sig(z)) with fixed shift M = sum|wa2| so
score blocks can be consumed in any order with a running denominator;
window pairs run middle-out as both directions' scores become available.
"""
import numpy as np

B, D, H, A = 128, 256, 256, 64
CH = 32

_BUILD_CACHE = {}
_last_in_maps = None


def _build(S):
    import concourse.bass as bass
    from concourse import bacc
    import concourse.mybir as mybir
    from concourse.tile import TileContext
    from concourse.masks import make_identity

    F32 = mybir.dt.float32
    BF16 = mybir.dt.bfloat16
    AF = mybir.ActivationFunctionType
    OP = mybir.AluOpType

    NCH = S // CH
    LAG = 2                    # L1 consumes L0 chunk j at tick j+LAG
    HLAG = 2                   # helper consumes L1 chunk j at tick j+LAG+HLAG
    MID = NCH // 2
    NW = NCH // 2              # middle-out window pairs
    WSTART = MID + LAG + HLAG + 1
    NT = WSTART + NW + 1       # + finalize tick

    G_H0 = [[0, 1], [2, 3], [4, 5], [6, 7]]
    G_H1 = [[1, 5], [3, 7], [0, 4], [2, 6]]
    G_AP = [[5, 7], [4, 6], [0, 2], [1, 3]]

    nc = bacc.Bacc("TRN2", target_bir_lowering=False, debug=False, num_devices=8)

    # ---- external inputs (same names across pids; content differs per pid)
    xs_d = nc.dram_tensor("xs", [S, 2, 128, B], BF16, kind="ExternalInput")
    wih_d = nc.dram_tensor("wihT", [128, 2, 8, 128], BF16, kind="ExternalInput")
    whh_d = nc.dram_tensor("whhT", [128, 2, 8, 128], BF16, kind="ExternalInput")
    bias_d = nc.dram_tensor("biasr", [2, 4, 128], BF16, kind="ExternalInput")
    hh_d = nc.dram_tensor("halfhot", [2, 512], BF16, kind="ExternalInput")
    wa1_d = nc.dram_tensor("wa1T", [128, 2, A], BF16, kind="ExternalInput")
    wa2_d = nc.dram_tensor("wa2c", [A, 1], BF16, kind="ExternalInput")
    mneg_d = nc.dram_tensor("mneg", [1, 1], F32, kind="ExternalInput")
    yout_d = nc.dram_tensor("yout", [B, H], F32, kind="ExternalOutput")

    # ---- internal DRAM (collective + staging)
    agi_h0 = [nc.dram_tensor(f"agi_h0_{j}", [CH, 2, 128, B], BF16) for j in range(NCH)]
    agb_h0 = [nc.dram_tensor(f"agb_h0_{j}", [2 * CH, 2, 128, B], BF16)
              for j in range(NCH)]
    agi_h1 = [nc.dram_tensor(f"agi_h1_{j}", [CH, 2, 128, B], BF16) for j in range(NCH)]
    agb_h1 = [nc.dram_tensor(f"agb_h1_{j}", [2 * CH, 2, 128, B], BF16)
              for j in range(NCH)]
    agi_ap = [nc.dram_tensor(f"agi_ap_{j}", [A, CH, B], BF16) for j in range(NCH)]
    agb_ap = [nc.dram_tensor(f"agb_ap_{j}", [2 * A, CH, B], BF16) for j in range(NCH)]
    hist_d = nc.dram_tensor("hist", [NCH, B, CH, H], BF16)
    # slot-asymmetric peer a_pre pull mirror (ring of 2 window ticks, 2 blocks)
    apmir = [nc.dram_tensor(f"apmir_{k}", [2, A, CH, B], BF16) for k in range(2)]

    with TileContext(nc) as tc:
        with tc.tile_pool(name="wpool", bufs=1) as wp, \
             tc.tile_pool(name="xpool", bufs=2) as xp, \
             tc.tile_pool(name="spool", bufs=2) as sp, \
             tc.tile_pool(name="apool", bufs=1) as ap_, \
             tc.tile_pool(name="psum", bufs=2, space="PSUM") as pp:

            pid = nc.partition_id()
            role = pid % 2 + (pid // 4) * 2
            # role 0: L0 recurrence; role 1: L1 recurrence;
            # role 3: attention helper; role 2: idle.

            # ---------------- prologue: constants and state ----------------
            wih = wp.tile([128, 2, 8, 128], BF16, tag="wih")
            nc.gpsimd.dma_start(out=wih[:], in_=wih_d.ap())
            whh = wp.tile([128, 2, 8, 128], BF16, tag="whh")
            nc.gpsimd.dma_start(out=whh[:], in_=whh_d.ap())
            biasr = wp.tile([2, 4, 128], BF16, tag="biasr")
            nc.gpsimd.dma_start(out=biasr[:], in_=bias_d.ap())
            halfhot = wp.tile([2, 512], BF16, tag="halfhot")
            nc.gpsimd.dma_start(out=halfhot[:], in_=hh_d.ap())
            wa1 = wp.tile([128, 2, A], BF16, tag="wa1")
            nc.gpsimd.dma_start(out=wa1[:], in_=wa1_d.ap())
            wa2 = wp.tile([A, 1], BF16, tag="wa2")
            nc.gpsimd.dma_start(out=wa2[:], in_=wa2_d.ap())
            mneg = wp.tile([128, 1], F32, tag="mneg")
            nc.sync.dma_start(out=mneg[:], in_=mneg_d.ap().partition_broadcast(128))
            ident = wp.tile([128, 128], F32, tag="ident")
            make_identity(nc, ident[:])
            identb = wp.tile([128, 128], BF16, tag="identb")
            nc.vector.tensor_copy(out=identb[:], in_=ident[:])

            # recurrence state (shared across roles; one role runs per core)
            c_s = wp.tile([128, 2, B], F32, tag="c_s")
            nc.gpsimd.memset(c_s[:], 0.0)
            # persistent prev-step h^T for the chunk boundary (reading the
            # previous chunk's pool tile from the next If-block deadlocks:
            # the block's slot allocation would wait on its own readers)
            hTp = wp.tile([128, 2, B], BF16, tag="hTp")
            nc.gpsimd.memset(hTp[:], 0.0)

            # attention state
            acc_v = wp.tile([128, 256], F32, tag="acc_v")
            nc.gpsimd.memset(acc_v[:], 0.0)
            den = wp.tile([128, 1], F32, tag="den")
            nc.gpsimd.memset(den[:], 0.0)

            # per-role input-chunk ping-pong tiles
            xt_pp = {
                0: (wp.tile([128, 2, CH, B], BF16, tag="xtA0", name="xtA0"),
                    wp.tile([128, 2, CH, B], BF16, tag="xtB0", name="xtB0")),
                1: (wp.tile([128, 2, CH, B], BF16, tag="xtA1", name="xtA1"),
                    wp.tile([128, 2, CH, B], BF16, tag="xtB1", name="xtB1")),
            }
            ht_pp = (wp.tile([128, 2, CH, B], BF16, tag="htA", name="htA"),
                     wp.tile([128, 2, CH, B], BF16, tag="htB", name="htB"))

            def pull_chunk(xt, srct, t0):
                # DMA supports <= 3 dims: split the [p,k,t,b] pull per kc
                for kc in range(2):
                    nc.sync.dma_start(
                        out=xt[:, kc],
                        in_=srct.ap()[t0:t0 + CH, kc]
                        .rearrange("t p b -> p t b"))

            # role0's chunk-0 input is ready in DRAM at start: prefetch now
            pull_chunk(xt_pp[0][0], xs_d, 0)

            def emit_xt_prefetch(cj, is_l1):
                xt = xt_pp[1 if is_l1 else 0][cj % 2]
                if is_l1:
                    pull_chunk(xt, agb_h0[cj], 0)
                else:
                    pull_chunk(xt, xs_d, cj * CH)


            def emit_group(xt, g, sfx):
                """bias + x@Wih prefill for steps 2g, 2g+1."""
                G = pp.tile([128, 4, 2, 2, B], F32, tag="G", name=f"G{sfx}_{g}")
                for bank in range(4):
                    nc.tensor.matmul(
                        G[:, bank].rearrange("p a t b -> p (a t b)"),
                        biasr[:, bank], halfhot[:],
                        start=True, stop=False, skip_group_check=True)
                for kc in range(2):
                    mov = xt[:, kc, 2 * g:2 * g + 2].rearrange("p t b -> p (t b)")
                    for gc in range(8):
                        nc.tensor.matmul(
                            G[:, gc // 2, gc % 2].rearrange("p t b -> p (t b)"),
                            wih[:, kc, gc], mov,
                            start=False, stop=False, skip_group_check=True)
                return G

            # ---------------- LSTM chunk (roles 0 and 1) ----------------
            def emit_lstm_chunk(cj, is_l1, sfx):
                rk = 1 if is_l1 else 0
                xt = xt_pp[rk][cj % 2]
                if cj + 1 < NCH:
                    emit_xt_prefetch(cj + 1, is_l1)
                hTc = xp.tile([128, CH, 2, B], BF16, tag="hTc", name=f"hTc{sfx}")

                def emit_step(G, t):
                    if t == 0:
                        prevT = hTp[:]
                    else:
                        prevT = hTc[:, t - 1]
                    for gc in range(8):
                        for kc in range(2):
                            nc.tensor.matmul(
                                G[:, gc // 2, gc % 2, t % 2],
                                whh[:, kc, gc], prevT[:, kc],
                                start=False, stop=(gc == 7 and kc == 1),
                                skip_group_check=True)
                    # gates: banks 0:i 1:f 2:g 3:o ; sg layout [p, gate, half, b]
                    sg = sp.tile([128, 4, 2, B], BF16, tag="sg", name=f"sg{sfx}_{t}")
                    nc.scalar.activation(sg[:, 0:2], G[:, 0:2, :, t % 2], AF.Sigmoid)
                    nc.scalar.activation(sg[:, 2], G[:, 2, :, t % 2], AF.Tanh)
                    nc.scalar.activation(sg[:, 3], G[:, 3, :, t % 2], AF.Sigmoid)
                    th = sp.tile([128, 2, B], BF16, tag="th", name=f"th{sfx}_{t}")
                    for h2 in range(2):
                        cf = sp.tile([128, B], F32, tag=f"cf{h2}",
                                     name=f"cf{sfx}_{t}_{h2}")
                        nc.vector.tensor_mul(out=cf[:], in0=c_s[:, h2],
                                             in1=sg[:, 1, h2])
                        v = sp.tile([128, B], F32, tag=f"v{h2}",
                                    name=f"v{sfx}_{t}_{h2}")
                        nc.vector.tensor_mul(out=v[:], in0=sg[:, 0, h2],
                                             in1=sg[:, 2, h2])
                        nc.vector.tensor_add(out=c_s[:, h2], in0=cf[:], in1=v[:])
                        nc.scalar.activation(th[:, h2], c_s[:, h2], AF.Tanh)
                        nc.vector.tensor_mul(out=hTc[:, t, h2], in0=sg[:, 3, h2],
                                             in1=th[:, h2])

                Gs = [emit_group(xt, 0, sfx), None]
                for t in range(CH):
                    if t % 2 == 0 and t + 2 < CH:
                        Gs[1] = emit_group(xt, t // 2 + 1, sfx)
                    emit_step(Gs[0], t)
                    if t % 2 == 1:
                        Gs[0] = Gs[1]

                # hand the last h^T to the next chunk via the persistent tile
                nc.vector.tensor_copy(out=hTp[:].rearrange("p k b -> p (k b)"),
                                      in_=hTc[:, CH - 1].rearrange(
                                          "p k b -> p (k b)"))
                # ship the chunk (per kc: DMA <= 3 dims)
                dst = agi_h1[cj] if is_l1 else agi_h0[cj]
                for kc in range(2):
                    nc.sync.dma_start(
                        out=dst.ap()[:, kc].rearrange("t p b -> p t b"),
                        in_=hTc[:, :, kc])

            # ---------------- helper: per-chunk a_pre + history ----------------
            def emit_helper_chunk(cj, sfx):
                ht = ht_pp[cj % 2]
                if cj + 1 < NCH:
                    for kc in range(2):
                        nc.sync.dma_start(
                            out=ht_pp[(cj + 1) % 2][:, kc],
                            in_=agb_h1[cj + 1].ap()[0:CH, kc]
                            .rearrange("t p b -> p t b"))
                # a_pre = Wa1_half^T @ h1 : [A, CH*B] in 2 passes of 16 t
                apo = ap_.tile([A, CH, B], BF16, tag="apo", name=f"apo{sfx}")
                for half in range(2):
                    aps = pp.tile([A, 4, 512], F32, tag="G", name=f"aps{sfx}_{half}")
                    for seg in range(4):
                        t0 = half * 16 + seg * 4
                        for kc in range(2):
                            nc.tensor.matmul(
                                aps[:, seg],
                                wa1[:, kc],
                                ht[:, kc, t0:t0 + 4].rearrange("p t b -> p (t b)"),
                                start=(kc == 0), stop=(kc == 1))
                    nc.vector.tensor_copy(
                        out=apo[:, half * 16:half * 16 + 16]
                        .rearrange("p t b -> p (t b)"),
                        in_=aps[:].rearrange("p s f -> p (s f)"))
                nc.sync.dma_start(out=agi_ap[cj].ap(), in_=apo[:])

                # history: transpose h^T -> [B, CH, H] and stage to DRAM
                hbl = ap_.tile([B, CH, H], BF16, tag="hbl", name=f"hbl{sfx}")
                for t in range(CH):
                    # [128, 2, 8, B] bf16 = 2 banks; kc slices land in
                    # different banks (transpose start=True zeroes a bank)
                    TP = pp.tile([128, 2, 8, B], BF16, tag="G", name=f"tp{sfx}_{t}")
                    for kc in range(2):
                        nc.tensor.transpose(TP[:, kc, 0], ht[:, kc, t], identb[:])
                    nc.vector.tensor_copy(
                        out=hbl[:, t].rearrange("p (k b) -> p k b", k=2),
                        in_=TP[:, :, 0])
                nc.sync.dma_start(out=hist_d.ap()[cj], in_=hbl[:])

            # ---------------- helper: window (score + weighted accum) ----------
            def emit_window(w, sfx):
                for bi in range(2):
                    cb = (MID - 1 - w) if bi == 0 else (MID + w)
                    apown = ap_.tile([A, CH, B], BF16, tag="apown",
                                     name=f"apw{sfx}_{bi}")
                    nc.sync.dma_start(out=apown[:], in_=agi_ap[cb].ap())
                    aprem = ap_.tile([A, CH, B], BF16, tag="aprem",
                                     name=f"apr{sfx}_{bi}")
                    nc.sync.dma_start(out=aprem[:], in_=apmir[w % 2].ap()[bi])
                    nc.vector.tensor_add(out=apown[:], in0=apown[:],
                                         in1=aprem[:][:, ::-1, :])
                    nc.scalar.activation(apown[:], apown[:], AF.Tanh)
                    # s[b, t] = sum_a wa2[a] * tanh[a, t, b]  (stationary = tanh_t)
                    spsum = pp.tile([128, 4, 2, 2, B], F32, tag="G",
                                    name=f"sp{sfx}_{bi}")
                    for t in range(CH):
                        nc.tensor.matmul(spsum[:, 0, 0, 0, t:t + 1],
                                         apown[:, t], wa2[:],
                                         start=(t == 0), stop=(t == CH - 1))
                    sco = ap_.tile([128, CH], F32, tag="sco", name=f"sc{sfx}_{bi}")
                    nc.scalar.activation(sco[:], spsum[:, 0, 0, 0, 0:CH],
                                         AF.Sigmoid, bias=mneg[:, 0:1])
                    dtl = ap_.tile([128, CH], F32, tag="dtl", name=f"dt{sfx}_{bi}")
                    nc.vector.tensor_scalar(out=dtl[:], in0=sco[:],
                                            scalar1=-1.0, scalar2=1.0,
                                            op0=OP.mult, op1=OP.add)
                    nc.vector.reciprocal(out=dtl[:], in_=dtl[:])
                    e_blk = ap_.tile([128, CH], F32, tag="e_blk",
                                     name=f"eb{sfx}_{bi}")
                    dinc = ap_.tile([128, 1], F32, tag="dinc", name=f"di{sfx}_{bi}")
                    nc.vector.scalar_tensor_tensor(
                        out=e_blk[:], in0=sco[:], scalar=1.0, in1=dtl[:],
                        op0=OP.mult, op1=OP.mult, accum_out=dinc[:])
                    nc.vector.tensor_add(out=den[:], in0=den[:], in1=dinc[:])
                    hw_ = ap_.tile([B, CH, H], BF16, tag="hbl", name=f"hw{sfx}_{bi}")
                    nc.sync.dma_start(out=hw_[:], in_=hist_d.ap()[cb])
                    for u in range(CH):
                        nc.vector.scalar_tensor_tensor(
                            out=acc_v[:], in0=hw_[:, u], scalar=e_blk[:, u:u + 1],
                            in1=acc_v[:], op0=OP.mult, op1=OP.add)

            # ---------------- tick loop ----------------
            for tick in range(NT):
                if tick < NCH:
                    with tc.If(role == 0, name=f"L0t{tick}"):
                        emit_lstm_chunk(tick, False, f"a{tick}")

                if tick == 1:
                    with tc.If(role == 1, name="L1pf"):
                        emit_xt_prefetch(0, True)

                j1 = tick - LAG
                if 0 <= j1 < NCH:
                    with tc.If(role == 1, name=f"L1t{tick}"):
                        emit_lstm_chunk(j1, True, f"b{tick}")

                j2 = tick - LAG - HLAG
                if j2 == -1:
                    with tc.If(role == 3, name="Hpf"):
                        for kc in range(2):
                            nc.sync.dma_start(
                                out=ht_pp[0][:, kc],
                                in_=agb_h1[0].ap()[0:CH, kc]
                                .rearrange("t p b -> p t b"))

                w = tick - WSTART
                # slot-asymmetric peer a_pre pulls into the apmir ring
                if 0 <= w < NW:
                    for slot, cpid in ((1, 5), (0, 7)):
                        with tc.If(pid == cpid, name=f"AR{tick}_{cpid}"):
                            for bi in range(2):
                                cb = (MID - 1 - w) if bi == 0 else (MID + w)
                                crem = NCH - 1 - cb
                                nc.sync.dma_start(
                                    out=apmir[w % 2].ap()[bi],
                                    in_=agb_ap[crem].ap()
                                    [slot * A:(slot + 1) * A])

                do_h = 0 <= j2 < NCH
                do_w = 0 <= w < NW
                if do_h or do_w or tick == NT - 1:
                    with tc.If(role == 3, name=f"Ht{tick}"):
                        if do_h:
                            emit_helper_chunk(j2, f"h{tick}")
                        if do_w:
                            emit_window(w, f"w{tick}")
                        if tick == NT - 1:
                            rden = ap_.tile([128, 1], F32, tag="dinc", name="rden")
                            nc.vector.reciprocal(out=rden[:], in_=den[:])
                            yt = ap_.tile([128, 256], F32, tag="yt", name="yt")
                            nc.vector.tensor_scalar_mul(yt[:], acc_v[:],
                                                        rden[:, 0:1])
                            nc.sync.dma_start(out=yout_d.ap(), in_=yt[:])

                # ---------------- collectives ----------------
                if tick < NCH:
                    nc.gpsimd.collective_compute(
                        "AllGather", mybir.AluOpType.bypass,
                        replica_groups=G_H0,
                        ins=[agi_h0[tick].ap()], outs=[agb_h0[tick].ap()])
                if 0 <= j1 < NCH:
                    nc.gpsimd.collective_compute(
                        "AllGather", mybir.AluOpType.bypass,
                        replica_groups=G_H1,
                        ins=[agi_h1[j1].ap()], outs=[agb_h1[j1].ap()])
                if 0 <= j2 < NCH:
                    nc.gpsimd.collective_compute(
                        "AllGather", mybir.AluOpType.bypass,
                        replica_groups=G_AP,
                        ins=[agi_ap[j2].ap()], outs=[agb_ap[j2].ap()])

    nc.compile()
    return nc


def _prep_lstm_w(Wih, Whh, bih, bhh, bf16):
    # torch gate order i,f,g,o matches the device gc order directly.
    def tile_w(M):
        # [1024, K] -> [128(p=k%128), 2(kc), 8(gc), 128(c)]
        return np.ascontiguousarray(
            np.asarray(M, np.float32).T.reshape(2, 128, 8, 128)
            .transpose(1, 0, 2, 3)).astype(bf16)

    bias = (np.asarray(bih, np.float32) + np.asarray(bhh, np.float32))
    # biasr[r, bank, :] = bias[(2*bank + r)*128 : ...]
    biasr = np.ascontiguousarray(
        bias.reshape(4, 2, 128).transpose(1, 0, 2)).astype(bf16)
    return tile_w(Wih), tile_w(Whh), biasr


def kernel(**inputs):
    import ml_dtypes
    from concourse.bass_utils import run_bass_kernel_spmd

    bf16 = ml_dtypes.bfloat16
    x = np.asarray(inputs["x"], np.float32)
    Bv, S, Dv = x.shape
    if (S, "nc") not in _BUILD_CACHE:
        _BUILD_CACHE[(S, "nc")] = _build(S)
    nc = _BUILD_CACHE[(S, "nc")]

    xs_f = np.ascontiguousarray(x.transpose(1, 2, 0)).reshape(
        S, 2, 128, Bv).astype(bf16)
    xs_b = np.ascontiguousarray(x[:, ::-1].transpose(1, 2, 0)).reshape(
        S, 2, 128, Bv).astype(bf16)
    z_xs = np.zeros((S, 2, 128, Bv), bf16)
    zw = np.zeros((128, 2, 8, 128), bf16)
    zb = np.zeros((2, 4, 128), bf16)

    wf0 = _prep_lstm_w(inputs["Wih_f0"], inputs["Whh_f0"], inputs["bih_f0"],
                       inputs["bhh_f0"], bf16)
    wf1 = _prep_lstm_w(inputs["Wih_f1"], inputs["Whh_f1"], inputs["bih_f1"],
                       inputs["bhh_f1"], bf16)
    wb0 = _prep_lstm_w(inputs["Wih_b0"], inputs["Whh_b0"], inputs["bih_b0"],
                       inputs["bhh_b0"], bf16)
    wb1 = _prep_lstm_w(inputs["Wih_b1"], inputs["Whh_b1"], inputs["bih_b1"],
                       inputs["bhh_b1"], bf16)

    halfhot = np.zeros((2, 512), np.float32)
    halfhot[0, 0:256] = 1.0
    halfhot[1, 256:512] = 1.0
    halfhot = halfhot.astype(bf16)

    wa1 = np.asarray(inputs["Wa1"], np.float32)          # [A, 2H]
    wa2 = np.asarray(inputs["Wa2"], np.float32).reshape(A)

    def wa1half(cols):
        # [A, 256] -> [128(p), 2(kc), A]
        return np.ascontiguousarray(
            cols.T.reshape(2, 128, A).transpose(1, 0, 2)).astype(bf16)

    wa1f = wa1half(wa1[:, 0:H])
    wa1b = wa1half(wa1[:, H:2 * H])
    zwa1 = np.zeros((128, 2, A), bf16)
    wa2c = wa2.reshape(A, 1).astype(bf16)
    mconst = np.float32(np.abs(wa2).sum())
    mneg = np.full((1, 1), -mconst, np.float32)

    def imap(xs, w3, wa1t):
        wih, whh, biasr = w3
        return {"xs": xs, "wihT": wih, "whhT": whh, "biasr": biasr,
                "halfhot": halfhot, "wa1T": wa1t, "wa2c": wa2c, "mneg": mneg}

    zero3 = (zw, zw, zb)
    in_maps = [
        imap(xs_f, wf0, zwa1), imap(z_xs, wf1, zwa1),
        imap(xs_b, wb0, zwa1), imap(z_xs, wb1, zwa1),
        imap(z_xs, zero3, zwa1), imap(z_xs, zero3, wa1f),
        imap(z_xs, zero3, zwa1), imap(z_xs, zero3, wa1b),
    ]
    global _last_in_maps
    _last_in_maps = in_maps
    res = run_bass_kernel_spmd(nc, in_maps, core_ids=list(range(8)))
    out = np.concatenate([res.results[5]["yout"], res.results[7]["yout"]], axis=1)
    return out.astype(np.float32)


# revision 17
# speedup vs baseline: 1.5287x; 1.0766x over previous
"""BiLSTM (2-layer, bidirectional) encoder + attention pooling on 8 Trainium2 cores.

Topology (one SPMD program, roles selected by partition id):
  pid 0: layer-0 forward LSTM   (full batch 128)      role 0
  pid 2: layer-0 backward LSTM  (host-reversed x)     role 0
  pid 1: layer-1 forward LSTM                         role 1
  pid 3: layer-1 backward LSTM                        role 1
  pid 5: f-side attention (a_pre, scores, pooling)    role 3
  pid 7: b-side attention                             role 3
  pid 4, 6: idle (participate in collectives with garbage pairs)

LSTM inner loop uses a transposed-gate layout: gates live in PSUM as
[4H-on-partitions x batch-free] tiles (8 gc tiles of [128,128] per step;
bank b holds gate-chunk pair (2b, 2b+1) for a 2-step group, ping-ponged).
Per group the PE prefills each bank with bias (rank-2 outer-product
matmul against a half-hot moving tile) + x@Wih (batched N=256 matmuls);
the recurrent Whh matmuls accumulate on top (start=False), so no
separate adds are needed and the PE stays dense (HAM stays warm).
h^T is produced directly by the DVE h-mul (no per-step transposes).
Gate nonlinearities use Sigmoid and true Tanh (same ACT table set).

Communication per 32-step chunk (all bf16):
  AG_h0 groups [[0,1],[2,3],..]: L0 hidden chunks -> L1 (lag 2 ticks).
  AG_h1 groups [[1,5],[3,7],..]: L1 hidden chunks -> attention helpers.
  AG_ap groups [[5,7],..]: A=64 attention pre-projections exchanged
      between the two helpers.
softmax uses exp(z) = sig(z)/(1-sig(z)) with fixed shift M = sum|wa2| so
score blocks can be consumed in any order with a running denominator;
window pairs run middle-out as both directions' scores become available.
"""
import numpy as np

B, D, H, A = 128, 256, 256, 64
CH = 32

_BUILD_CACHE = {}
_last_in_maps = None


def _build(S):
    import concourse.bass as bass
    from concourse import bacc
    import concourse.mybir as mybir
    from concourse.tile import TileContext
    from concourse.masks import make_identity

    F32 = mybir.dt.float32
    BF16 = mybir.dt.bfloat16
    AF = mybir.ActivationFunctionType
    OP = mybir.AluOpType

    NCH = S // CH
    LAG = 1                    # L1 consumes L0 chunk j at tick j+LAG
    HLAG = 1                   # helper consumes L1 chunk j at tick j+LAG+HLAG
    MID = NCH // 2
    NW = NCH // 2              # middle-out window pairs
    WSTART = MID + LAG + HLAG + 1
    NT = WSTART + NW + 1       # + finalize tick

    G_H0 = [[0, 1], [2, 3], [4, 5], [6, 7]]
    G_H1 = [[1, 5], [3, 7], [0, 4], [2, 6]]
    G_AP = [[5, 7], [4, 6], [0, 2], [1, 3]]

    nc = bacc.Bacc("TRN2", target_bir_lowering=False, debug=False, num_devices=8)

    # ---- external inputs (same names across pids; content differs per pid)
    xs_d = nc.dram_tensor("xs", [S, 2, 128, B], BF16, kind="ExternalInput")
    wih_d = nc.dram_tensor("wihT", [128, 2, 8, 128], BF16, kind="ExternalInput")
    whh_d = nc.dram_tensor("whhT", [128, 2, 8, 128], BF16, kind="ExternalInput")
    bias_d = nc.dram_tensor("biasq", [4, 2, 128], BF16, kind="ExternalInput")
    hh_d = nc.dram_tensor("qhot", [4, 512], BF16, kind="ExternalInput")
    wa1_d = nc.dram_tensor("wa1T", [128, 2, A], BF16, kind="ExternalInput")
    wa2_d = nc.dram_tensor("wa2c", [A, 1], BF16, kind="ExternalInput")
    mneg_d = nc.dram_tensor("mneg", [1, 1], F32, kind="ExternalInput")
    yout_d = nc.dram_tensor("yout", [B, H], F32, kind="ExternalOutput")

    # ---- internal DRAM (collective + staging)
    agi_h0 = [nc.dram_tensor(f"agi_h0_{j}", [CH, 2, 128, B], BF16) for j in range(NCH)]
    agb_h0 = [nc.dram_tensor(f"agb_h0_{j}", [2 * CH, 2, 128, B], BF16)
              for j in range(NCH)]
    agi_h1 = [nc.dram_tensor(f"agi_h1_{j}", [CH, 2, 128, B], BF16) for j in range(NCH)]
    agb_h1 = [nc.dram_tensor(f"agb_h1_{j}", [2 * CH, 2, 128, B], BF16)
              for j in range(NCH)]
    agi_ap = [nc.dram_tensor(f"agi_ap_{j}", [A, CH, B], BF16) for j in range(NCH)]
    agb_ap = [nc.dram_tensor(f"agb_ap_{j}", [2 * A, CH, B], BF16) for j in range(NCH)]
    hist_d = nc.dram_tensor("hist", [NCH, B, CH, H], BF16)
    # slot-asymmetric peer a_pre pull mirror (ring of 2 window ticks, 2 blocks)
    apmir = [nc.dram_tensor(f"apmir_{k}", [2, A, CH, B], BF16) for k in range(2)]

    with TileContext(nc) as tc:
        with tc.tile_pool(name="wpool", bufs=1) as wp, \
             tc.tile_pool(name="xpool", bufs=2) as xp, \
             tc.tile_pool(name="spool", bufs=2) as sp, \
             tc.tile_pool(name="apool", bufs=1) as ap_, \
             tc.tile_pool(name="psum", bufs=4, space="PSUM") as pp:

            pid = nc.partition_id()
            role = pid % 2 + (pid // 4) * 2
            # role 0: L0 recurrence; role 1: L1 recurrence;
            # role 3: attention helper; role 2: idle.

            # ---------------- prologue: constants and state ----------------
            wih = wp.tile([128, 2, 8, 128], BF16, tag="wih")
            nc.gpsimd.dma_start(out=wih[:], in_=wih_d.ap())
            whh = wp.tile([128, 2, 8, 128], BF16, tag="whh")
            nc.gpsimd.dma_start(out=whh[:], in_=whh_d.ap())
            biasq = wp.tile([4, 2, 128], BF16, tag="biasq")
            nc.gpsimd.dma_start(out=biasq[:], in_=bias_d.ap())
            qhot = wp.tile([4, 512], BF16, tag="qhot")
            nc.gpsimd.dma_start(out=qhot[:], in_=hh_d.ap())
            wa1 = wp.tile([128, 2, A], BF16, tag="wa1")
            nc.gpsimd.dma_start(out=wa1[:], in_=wa1_d.ap())
            wa2 = wp.tile([A, 1], BF16, tag="wa2")
            nc.gpsimd.dma_start(out=wa2[:], in_=wa2_d.ap())
            mneg = wp.tile([128, 1], F32, tag="mneg")
            nc.sync.dma_start(out=mneg[:], in_=mneg_d.ap().partition_broadcast(128))
            ident = wp.tile([128, 128], F32, tag="ident")
            make_identity(nc, ident[:])
            identb = wp.tile([128, 128], BF16, tag="identb")
            nc.vector.tensor_copy(out=identb[:], in_=ident[:])

            # recurrence state (shared across roles; one role runs per core)
            c_s = wp.tile([128, 2, B], F32, tag="c_s")
            nc.gpsimd.memset(c_s[:], 0.0)
            # persistent prev-step h^T for the chunk boundary (reading the
            # previous chunk's pool tile from the next If-block deadlocks:
            # the block's slot allocation would wait on its own readers)
            hTp = wp.tile([128, 2, B], BF16, tag="hTp")
            nc.gpsimd.memset(hTp[:], 0.0)

            # attention state
            acc_v = wp.tile([128, 256], F32, tag="acc_v")
            nc.gpsimd.memset(acc_v[:], 0.0)
            den = wp.tile([128, 1], F32, tag="den")
            nc.gpsimd.memset(den[:], 0.0)

            # per-role input-chunk ping-pong tiles
            xt_pp = {
                0: (wp.tile([128, 2, CH, B], BF16, tag="xtA0", name="xtA0"),
                    wp.tile([128, 2, CH, B], BF16, tag="xtB0", name="xtB0")),
                1: (wp.tile([128, 2, CH, B], BF16, tag="xtA1", name="xtA1"),
                    wp.tile([128, 2, CH, B], BF16, tag="xtB1", name="xtB1")),
            }
            ht_pp = (wp.tile([128, 2, CH, B], BF16, tag="htA", name="htA"),
                     wp.tile([128, 2, CH, B], BF16, tag="htB", name="htB"))

            def pull_chunk(xt, srct, t0):
                # DMA supports <= 3 dims: split the [p,k,t,b] pull per kc
                for kc in range(2):
                    nc.sync.dma_start(
                        out=xt[:, kc],
                        in_=srct.ap()[t0:t0 + CH, kc]
                        .rearrange("t p b -> p t b"))

            # role0's chunk-0 input is ready in DRAM at start: prefetch now
            pull_chunk(xt_pp[0][0], xs_d, 0)

            def emit_xt_prefetch(cj, is_l1):
                xt = xt_pp[1 if is_l1 else 0][cj % 2]
                if is_l1:
                    pull_chunk(xt, agb_h0[cj], 0)
                else:
                    pull_chunk(xt, xs_d, cj * CH)


            def emit_group(xt, t, sfx):
                """bias + x@Wih prefill for step t (2 PSUM banks)."""
                G = pp.tile([128, 2, 4, B], F32, tag="G", name=f"G{sfx}_{t}")
                for bank in range(2):
                    nc.tensor.matmul(
                        G[:, bank].rearrange("p a b -> p (a b)"),
                        biasq[:, bank], qhot[:],
                        start=True, stop=False, skip_group_check=True)
                for kc in range(2):
                    mov = xt[:, kc, t]
                    for gc in range(8):
                        nc.tensor.matmul(
                            G[:, gc // 4, gc % 4],
                            wih[:, kc, gc], mov,
                            start=False, stop=False, skip_group_check=True)
                return G

            # ---------------- LSTM chunk (roles 0 and 1) ----------------
            def emit_lstm_chunk(cj, is_l1, sfx):
                rk = 1 if is_l1 else 0
                xt = xt_pp[rk][cj % 2]
                if cj + 1 < NCH:
                    emit_xt_prefetch(cj + 1, is_l1)
                hTc = xp.tile([128, CH, 2, B], BF16, tag="hTc", name=f"hTc{sfx}")

                def emit_step(G, t):
                    if t == 0:
                        prevT = hTp[:]
                    else:
                        prevT = hTc[:, t - 1]
                    for gc in range(8):
                        for kc in range(2):
                            nc.tensor.matmul(
                                G[:, gc // 4, gc % 4],
                                whh[:, kc, gc], prevT[:, kc],
                                start=False, stop=(gc == 7 and kc == 1),
                                skip_group_check=True)
                    # gates: gc 0-1:i 2-3:f 4-5:g 6-7:o ; sg = [p, gate, half, b]
                    sg = sp.tile([128, 4, 2, B], BF16, tag="sg", name=f"sg{sfx}_{t}")
                    nc.scalar.activation(sg[:, 0:2], G[:, 0], AF.Sigmoid)
                    nc.scalar.activation(sg[:, 2], G[:, 1, 0:2], AF.Tanh)
                    nc.scalar.activation(sg[:, 3], G[:, 1, 2:4], AF.Sigmoid)
                    th = sp.tile([128, 2, B], BF16, tag="th", name=f"th{sfx}_{t}")
                    cf = sp.tile([128, 2, B], F32, tag="cf", name=f"cf{sfx}_{t}")
                    nc.vector.tensor_mul(out=cf[:], in0=c_s[:], in1=sg[:, 1])
                    v = sp.tile([128, 2, B], F32, tag="v", name=f"v{sfx}_{t}")
                    nc.vector.tensor_mul(out=v[:], in0=sg[:, 0], in1=sg[:, 2])
                    nc.vector.tensor_add(out=c_s[:], in0=cf[:], in1=v[:])
                    nc.scalar.activation(th[:], c_s[:], AF.Tanh)
                    nc.vector.tensor_mul(out=hTc[:, t], in0=sg[:, 3], in1=th[:])

                Gs = [emit_group(xt, 0, sfx), emit_group(xt, 1, sfx),
                      emit_group(xt, 2, sfx)]
                for t in range(CH):
                    if t + 3 < CH:
                        Gs.append(emit_group(xt, t + 3, sfx))
                    emit_step(Gs[t], t)

                # hand the last h^T to the next chunk via the persistent tile
                nc.vector.tensor_copy(out=hTp[:].rearrange("p k b -> p (k b)"),
                                      in_=hTc[:, CH - 1].rearrange(
                                          "p k b -> p (k b)"))
                # ship the chunk (per kc: DMA <= 3 dims)
                dst = agi_h1[cj] if is_l1 else agi_h0[cj]
                for kc in range(2):
                    nc.sync.dma_start(
                        out=dst.ap()[:, kc].rearrange("t p b -> p t b"),
                        in_=hTc[:, :, kc])

            # ---------------- helper: per-chunk a_pre + history ----------------
            def emit_helper_chunk(cj, sfx):
                ht = ht_pp[cj % 2]
                if cj + 1 < NCH:
                    for kc in range(2):
                        nc.sync.dma_start(
                            out=ht_pp[(cj + 1) % 2][:, kc],
                            in_=agb_h1[cj + 1].ap()[0:CH, kc]
                            .rearrange("t p b -> p t b"))
                # a_pre = Wa1_half^T @ h1 : [A, CH*B] in 4 passes of 8 t
                apo = ap_.tile([A, CH, B], BF16, tag="apo", name=f"apo{sfx}")
                for qu in range(4):
                    aps = pp.tile([A, 2, 512], F32, tag="G", name=f"aps{sfx}_{qu}")
                    for seg in range(2):
                        t0 = qu * 8 + seg * 4
                        for kc in range(2):
                            nc.tensor.matmul(
                                aps[:, seg],
                                wa1[:, kc],
                                ht[:, kc, t0:t0 + 4].rearrange("p t b -> p (t b)"),
                                start=(kc == 0), stop=(kc == 1))
                    nc.vector.tensor_copy(
                        out=apo[:, qu * 8:qu * 8 + 8]
                        .rearrange("p t b -> p (t b)"),
                        in_=aps[:].rearrange("p s f -> p (s f)"))
                nc.sync.dma_start(out=agi_ap[cj].ap(), in_=apo[:])

                # history: transpose h^T -> [B, CH, H] and stage to DRAM
                hbl = ap_.tile([B, CH, H], BF16, tag="hbl", name=f"hbl{sfx}")
                for t in range(CH):
                    # [128, 2, 8, B] bf16 = 2 banks; kc slices land in
                    # different banks (transpose start=True zeroes a bank)
                    TP = pp.tile([128, 2, 8, B], BF16, tag="G", name=f"tp{sfx}_{t}")
                    for kc in range(2):
                        nc.tensor.transpose(TP[:, kc, 0], ht[:, kc, t], identb[:])
                    nc.vector.tensor_copy(
                        out=hbl[:, t].rearrange("p (k b) -> p k b", k=2),
                        in_=TP[:, :, 0])
                nc.sync.dma_start(out=hist_d.ap()[cj], in_=hbl[:])

            # ---------------- helper: window (score + weighted accum) ----------
            def emit_window(w, sfx):
                for bi in range(2):
                    cb = (MID - 1 - w) if bi == 0 else (MID + w)
                    apown = ap_.tile([A, CH, B], BF16, tag="apown",
                                     name=f"apw{sfx}_{bi}")
                    nc.sync.dma_start(out=apown[:], in_=agi_ap[cb].ap())
                    aprem = ap_.tile([A, CH, B], BF16, tag="aprem",
                                     name=f"apr{sfx}_{bi}")
                    nc.sync.dma_start(out=aprem[:], in_=apmir[w % 2].ap()[bi])
                    nc.vector.tensor_add(out=apown[:], in0=apown[:],
                                         in1=aprem[:][:, ::-1, :])
                    nc.scalar.activation(apown[:], apown[:], AF.Tanh)
                    # s[b, t] = sum_a wa2[a] * tanh[a, t, b]  (stationary = tanh_t)
                    spsum = pp.tile([128, 2, 4, B], F32, tag="G",
                                    name=f"sp{sfx}_{bi}")
                    for t in range(CH):
                        nc.tensor.matmul(spsum[:, 0, 0, t:t + 1],
                                         apown[:, t], wa2[:],
                                         start=(t == 0), stop=(t == CH - 1))
                    sco = ap_.tile([128, CH], F32, tag="sco", name=f"sc{sfx}_{bi}")
                    nc.scalar.activation(sco[:], spsum[:, 0, 0, 0:CH],
                                         AF.Sigmoid, bias=mneg[:, 0:1])
                    dtl = ap_.tile([128, CH], F32, tag="dtl", name=f"dt{sfx}_{bi}")
                    nc.vector.tensor_scalar(out=dtl[:], in0=sco[:],
                                            scalar1=-1.0, scalar2=1.0,
                                            op0=OP.mult, op1=OP.add)
                    nc.vector.reciprocal(out=dtl[:], in_=dtl[:])
                    e_blk = ap_.tile([128, CH], F32, tag="e_blk",
                                     name=f"eb{sfx}_{bi}")
                    dinc = ap_.tile([128, 1], F32, tag="dinc", name=f"di{sfx}_{bi}")
                    nc.vector.scalar_tensor_tensor(
                        out=e_blk[:], in0=sco[:], scalar=1.0, in1=dtl[:],
                        op0=OP.mult, op1=OP.mult, accum_out=dinc[:])
                    nc.vector.tensor_add(out=den[:], in0=den[:], in1=dinc[:])
                    hw_ = ap_.tile([B, CH, H], BF16, tag="hbl", name=f"hw{sfx}_{bi}")
                    nc.sync.dma_start(out=hw_[:], in_=hist_d.ap()[cb])
                    for u in range(CH):
                        nc.vector.scalar_tensor_tensor(
                            out=acc_v[:], in0=hw_[:, u], scalar=e_blk[:, u:u + 1],
                            in1=acc_v[:], op0=OP.mult, op1=OP.add)

            # ---------------- tick loop ----------------
            for tick in range(NT):
                if tick < NCH:
                    with tc.If(role == 0, name=f"L0t{tick}"):
                        emit_lstm_chunk(tick, False, f"a{tick}")
                    nc.gpsimd.collective_compute(
                        "AllGather", mybir.AluOpType.bypass,
                        replica_groups=G_H0,
                        ins=[agi_h0[tick].ap()], outs=[agb_h0[tick].ap()])

                if tick == LAG:
                    with tc.If(role == 1, name="L1pf"):
                        emit_xt_prefetch(0, True)

                j1 = tick - LAG
                if 0 <= j1 < NCH:
                    with tc.If(role == 1, name=f"L1t{tick}"):
                        emit_lstm_chunk(j1, True, f"b{tick}")
                    nc.gpsimd.collective_compute(
                        "AllGather", mybir.AluOpType.bypass,
                        replica_groups=G_H1,
                        ins=[agi_h1[j1].ap()], outs=[agb_h1[j1].ap()])

                j2 = tick - LAG - HLAG
                if j2 == -1:
                    with tc.If(role == 3, name="Hpf"):
                        for kc in range(2):
                            nc.sync.dma_start(
                                out=ht_pp[0][:, kc],
                                in_=agb_h1[0].ap()[0:CH, kc]
                                .rearrange("t p b -> p t b"))

                w = tick - WSTART
                # slot-asymmetric peer a_pre pulls into the apmir ring
                if 0 <= w < NW:
                    for slot, cpid in ((1, 5), (0, 7)):
                        with tc.If(pid == cpid, name=f"AR{tick}_{cpid}"):
                            for bi in range(2):
                                cb = (MID - 1 - w) if bi == 0 else (MID + w)
                                crem = NCH - 1 - cb
                                nc.sync.dma_start(
                                    out=apmir[w % 2].ap()[bi],
                                    in_=agb_ap[crem].ap()
                                    [slot * A:(slot + 1) * A])

                do_h = 0 <= j2 < NCH
                do_w = 0 <= w < NW
                if do_h or do_w or tick == NT - 1:
                    with tc.If(role == 3, name=f"Ht{tick}"):
                        if do_h:
                            emit_helper_chunk(j2, f"h{tick}")
                        if do_w:
                            emit_window(w, f"w{tick}")
                        if tick == NT - 1:
                            rden = ap_.tile([128, 1], F32, tag="dinc", name="rden")
                            nc.vector.reciprocal(out=rden[:], in_=den[:])
                            yt = ap_.tile([128, 256], F32, tag="yt", name="yt")
                            nc.vector.tensor_scalar_mul(yt[:], acc_v[:],
                                                        rden[:, 0:1])
                            nc.sync.dma_start(out=yout_d.ap(), in_=yt[:])

                if 0 <= j2 < NCH:
                    nc.gpsimd.collective_compute(
                        "AllGather", mybir.AluOpType.bypass,
                        replica_groups=G_AP,
                        ins=[agi_ap[j2].ap()], outs=[agb_ap[j2].ap()])

    nc.compile()
    return nc


def _prep_lstm_w(Wih, Whh, bih, bhh, bf16):
    # torch gate order i,f,g,o matches the device gc order directly.
    def tile_w(M):
        # [1024, K] -> [128(p=k%128), 2(kc), 8(gc), 128(c)]
        return np.ascontiguousarray(
            np.asarray(M, np.float32).T.reshape(2, 128, 8, 128)
            .transpose(1, 0, 2, 3)).astype(bf16)

    bias = (np.asarray(bih, np.float32) + np.asarray(bhh, np.float32))
    # biasq[r, bank, :] = bias[(4*bank + r)*128 : ...]
    biasq = np.ascontiguousarray(
        bias.reshape(2, 4, 128).transpose(1, 0, 2)).astype(bf16)
    return tile_w(Wih), tile_w(Whh), biasq


def kernel(**inputs):
    import ml_dtypes
    from concourse.bass_utils import run_bass_kernel_spmd

    bf16 = ml_dtypes.bfloat16
    x = np.asarray(inputs["x"], np.float32)
    Bv, S, Dv = x.shape
    if (S, "nc") not in _BUILD_CACHE:
        _BUILD_CACHE[(S, "nc")] = _build(S)
    nc = _BUILD_CACHE[(S, "nc")]

    xs_f = np.ascontiguousarray(x.transpose(1, 2, 0)).reshape(
        S, 2, 128, Bv).astype(bf16)
    xs_b = np.ascontiguousarray(x[:, ::-1].transpose(1, 2, 0)).reshape(
        S, 2, 128, Bv).astype(bf16)
    z_xs = np.zeros((S, 2, 128, Bv), bf16)
    zw = np.zeros((128, 2, 8, 128), bf16)
    zb = np.zeros((4, 2, 128), bf16)

    wf0 = _prep_lstm_w(inputs["Wih_f0"], inputs["Whh_f0"], inputs["bih_f0"],
                       inputs["bhh_f0"], bf16)
    wf1 = _prep_lstm_w(inputs["Wih_f1"], inputs["Whh_f1"], inputs["bih_f1"],
                       inputs["bhh_f1"], bf16)
    wb0 = _prep_lstm_w(inputs["Wih_b0"], inputs["Whh_b0"], inputs["bih_b0"],
                       inputs["bhh_b0"], bf16)
    wb1 = _prep_lstm_w(inputs["Wih_b1"], inputs["Whh_b1"], inputs["bih_b1"],
                       inputs["bhh_b1"], bf16)

    qhot = np.zeros((4, 512), np.float32)
    for r in range(4):
        qhot[r, r * 128:(r + 1) * 128] = 1.0
    qhot = qhot.astype(bf16)

    wa1 = np.asarray(inputs["Wa1"], np.float32)          # [A, 2H]
    wa2 = np.asarray(inputs["Wa2"], np.float32).reshape(A)

    def wa1half(cols):
        # [A, 256] -> [128(p), 2(kc), A]
        return np.ascontiguousarray(
            cols.T.reshape(2, 128, A).transpose(1, 0, 2)).astype(bf16)

    wa1f = wa1half(wa1[:, 0:H])
    wa1b = wa1half(wa1[:, H:2 * H])
    zwa1 = np.zeros((128, 2, A), bf16)
    wa2c = wa2.reshape(A, 1).astype(bf16)
    mconst = np.float32(np.abs(wa2).sum())
    mneg = np.full((1, 1), -mconst, np.float32)

    def imap(xs, w3, wa1t):
        wih, whh, biasq = w3
        return {"xs": xs, "wihT": wih, "whhT": whh, "biasq": biasq,
                "qhot": qhot, "wa1T": wa1t, "wa2c": wa2c, "mneg": mneg}

    zero3 = (zw, zw, zb)
    in_maps = [
        imap(xs_f, wf0, zwa1), imap(z_xs, wf1, zwa1),
        imap(xs_b, wb0, zwa1), imap(z_xs, wb1, zwa1),
        imap(z_xs, zero3, zwa1), imap(z_xs, zero3, wa1f),
        imap(z_xs, zero3, zwa1), imap(z_xs, zero3, wa1b),
    ]
    global _last_in_maps
    _last_in_maps = in_maps
    res = run_bass_kernel_spmd(nc, in_maps, core_ids=list(range(8)))
    out = np.concatenate([res.results[5]["yout"], res.results[7]["yout"]], axis=1)
    return out.astype(np.float32)


# revision 18
# speedup vs baseline: 2.3237x; 1.5200x over previous
"""BiLSTM (2-layer, bidirectional) encoder + attention pooling on 8 Trainium2 cores.

Topology (one SPMD program, roles selected by partition id):
  pid 0: layer-0 forward LSTM   (full batch 128)      role 0
  pid 2: layer-0 backward LSTM  (host-reversed x)     role 0
  pid 1: layer-1 forward LSTM                         role 1
  pid 3: layer-1 backward LSTM                        role 1
  pid 5: f-side attention (a_pre, scores, pooling)    role 3
  pid 7: b-side attention                             role 3
  pid 4, 6: idle (participate in collectives with garbage pairs)

LSTM inner loop uses a transposed-gate layout: gates live in PSUM as
[4H-on-partitions x batch-free] tiles (8 gc tiles of [128,128] per step;
bank b holds gate-chunk pair (2b, 2b+1) for a 2-step group, ping-ponged).
Per group the PE prefills each bank with bias (rank-2 outer-product
matmul against a half-hot moving tile) + x@Wih (batched N=256 matmuls);
the recurrent Whh matmuls accumulate on top (start=False), so no
separate adds are needed and the PE stays dense (HAM stays warm).
h^T is produced directly by the DVE h-mul (no per-step transposes).
Gate nonlinearities use Sigmoid and true Tanh (same ACT table set).

Communication per 32-step chunk (all bf16):
  AG_h0 groups [[0,1],[2,3],..]: L0 hidden chunks -> L1 (lag 2 ticks).
  AG_h1 groups [[1,5],[3,7],..]: L1 hidden chunks -> attention helpers.
  AG_ap groups [[5,7],..]: A=64 attention pre-projections exchanged
      between the two helpers.
softmax uses exp(z) = sig(z)/(1-sig(z)) with fixed shift M = sum|wa2| so
score blocks can be consumed in any order with a running denominator;
window pairs run middle-out as both directions' scores become available.
"""
import numpy as np

B, D, H, A = 128, 256, 256, 64
CH = 32

_BUILD_CACHE = {}
_last_in_maps = None


def _build(S):
    import concourse.bass as bass
    from concourse import bacc
    import concourse.mybir as mybir
    from concourse.tile import TileContext
    from concourse.masks import make_identity

    F32 = mybir.dt.float32
    BF16 = mybir.dt.bfloat16
    AF = mybir.ActivationFunctionType
    OP = mybir.AluOpType

    NCH = S // CH
    LAG = 1                    # L1 consumes L0 chunk j at tick j+LAG
    HLAG = 1                   # helper consumes L1 chunk j at tick j+LAG+HLAG
    MID = NCH // 2
    NW = NCH // 2              # middle-out window pairs
    WSTART = MID + LAG + HLAG + 1
    NT = WSTART + NW + 1       # + finalize tick

    G_H0 = [[0, 1], [2, 3], [4, 5], [6, 7]]
    G_H1 = [[1, 5], [3, 7], [0, 4], [2, 6]]
    G_AP = [[5, 7], [4, 6], [0, 2], [1, 3]]

    nc = bacc.Bacc("TRN2", target_bir_lowering=False, debug=False, num_devices=8)

    # ---- external inputs (same names across pids; content differs per pid)
    xs_d = nc.dram_tensor("xs", [S, 2, 128, B], BF16, kind="ExternalInput")
    wih_d = nc.dram_tensor("wihT", [128, 2, 8, 128], BF16, kind="ExternalInput")
    whh_d = nc.dram_tensor("whhT", [128, 2, 8, 128], BF16, kind="ExternalInput")
    bias_d = nc.dram_tensor("biasq", [4, 2, 128], BF16, kind="ExternalInput")
    hh_d = nc.dram_tensor("qhot", [4, 512], BF16, kind="ExternalInput")
    wa1_d = nc.dram_tensor("wa1T", [128, 2, A], BF16, kind="ExternalInput")
    wa2_d = nc.dram_tensor("wa2c", [A, 1], BF16, kind="ExternalInput")
    mneg_d = nc.dram_tensor("mneg", [1, 1], F32, kind="ExternalInput")
    yout_d = nc.dram_tensor("yout", [B, H], F32, kind="ExternalOutput")

    # ---- internal DRAM (collective + staging)
    agi_h0 = [nc.dram_tensor(f"agi_h0_{j}", [CH, 2, 128, B], BF16) for j in range(NCH)]
    agb_h0 = [nc.dram_tensor(f"agb_h0_{j}", [2 * CH, 2, 128, B], BF16)
              for j in range(NCH)]
    agi_h1 = [nc.dram_tensor(f"agi_h1_{j}", [CH, 2, 128, B], BF16) for j in range(NCH)]
    agb_h1 = [nc.dram_tensor(f"agb_h1_{j}", [2 * CH, 2, 128, B], BF16)
              for j in range(NCH)]
    agi_ap = [nc.dram_tensor(f"agi_ap_{j}", [A, CH, B], BF16) for j in range(NCH)]
    agb_ap = [nc.dram_tensor(f"agb_ap_{j}", [2 * A, CH, B], BF16) for j in range(NCH)]
    hist_d = nc.dram_tensor("hist", [NCH, B, CH, H], BF16)
    # slot-asymmetric peer a_pre pull mirror (ring of 2 window ticks, 2 blocks)
    apmir = [nc.dram_tensor(f"apmir_{k}", [2, A, CH, B], BF16) for k in range(2)]

    with TileContext(nc) as tc:
        with tc.tile_pool(name="wpool", bufs=1) as wp, \
             tc.tile_pool(name="xpool", bufs=2) as xp, \
             tc.tile_pool(name="spool", bufs=2) as sp, \
             tc.tile_pool(name="apool", bufs=1) as ap_, \
             tc.tile_pool(name="psum", bufs=4, space="PSUM") as pp:

            pid = nc.partition_id()
            role = pid % 2 + (pid // 4) * 2
            # role 0: L0 recurrence; role 1: L1 recurrence;
            # role 3: attention helper; role 2: idle.

            # ---------------- prologue: constants and state ----------------
            wih = wp.tile([128, 2, 8, 128], BF16, tag="wih")
            nc.gpsimd.dma_start(out=wih[:], in_=wih_d.ap())
            whh = wp.tile([128, 2, 8, 128], BF16, tag="whh")
            nc.gpsimd.dma_start(out=whh[:], in_=whh_d.ap())
            biasq = wp.tile([4, 2, 128], BF16, tag="biasq")
            nc.gpsimd.dma_start(out=biasq[:], in_=bias_d.ap())
            qhot = wp.tile([4, 512], BF16, tag="qhot")
            nc.gpsimd.dma_start(out=qhot[:], in_=hh_d.ap())
            wa1 = wp.tile([128, 2, A], BF16, tag="wa1")
            nc.gpsimd.dma_start(out=wa1[:], in_=wa1_d.ap())
            wa2 = wp.tile([A, 1], BF16, tag="wa2")
            nc.gpsimd.dma_start(out=wa2[:], in_=wa2_d.ap())
            mneg = wp.tile([128, 1], F32, tag="mneg")
            nc.sync.dma_start(out=mneg[:], in_=mneg_d.ap().partition_broadcast(128))
            ident = wp.tile([128, 128], F32, tag="ident")
            make_identity(nc, ident[:])
            identb = wp.tile([128, 128], BF16, tag="identb")
            nc.vector.tensor_copy(out=identb[:], in_=ident[:])

            # recurrence state (shared across roles; one role runs per core)
            c_s = wp.tile([128, 2, B], BF16, tag="c_s")
            nc.gpsimd.memset(c_s[:], 0.0)
            # persistent prev-step h^T for the chunk boundary (reading the
            # previous chunk's pool tile from the next If-block deadlocks:
            # the block's slot allocation would wait on its own readers)
            hTp = wp.tile([128, 2, B], BF16, tag="hTp")
            nc.gpsimd.memset(hTp[:], 0.0)

            # attention state
            acc_v = wp.tile([128, 256], F32, tag="acc_v")
            nc.gpsimd.memset(acc_v[:], 0.0)
            den = wp.tile([128, 1], F32, tag="den")
            nc.gpsimd.memset(den[:], 0.0)

            # per-role input-chunk ping-pong tiles
            xt_pp = {
                0: (wp.tile([128, 2, CH, B], BF16, tag="xtA0", name="xtA0"),
                    wp.tile([128, 2, CH, B], BF16, tag="xtB0", name="xtB0")),
                1: (wp.tile([128, 2, CH, B], BF16, tag="xtA1", name="xtA1"),
                    wp.tile([128, 2, CH, B], BF16, tag="xtB1", name="xtB1")),
            }
            ht_pp = (wp.tile([128, 2, CH, B], BF16, tag="htA", name="htA"),
                     wp.tile([128, 2, CH, B], BF16, tag="htB", name="htB"))

            def pull_chunk(xt, srct, t0):
                # DMA supports <= 3 dims: split the [p,k,t,b] pull per kc
                for kc in range(2):
                    nc.sync.dma_start(
                        out=xt[:, kc],
                        in_=srct.ap()[t0:t0 + CH, kc]
                        .rearrange("t p b -> p t b"))

            # role0's chunk-0 input is ready in DRAM at start: prefetch now
            pull_chunk(xt_pp[0][0], xs_d, 0)

            def emit_xt_prefetch(cj, is_l1):
                xt = xt_pp[1 if is_l1 else 0][cj % 2]
                if is_l1:
                    pull_chunk(xt, agb_h0[cj], 0)
                else:
                    pull_chunk(xt, xs_d, cj * CH)


            def emit_group(xt, t, sfx):
                """bias + x@Wih prefill for step t; per-bank tiles so each
                bank's slot releases as soon as its own ACT read completes."""
                G1 = pp.tile([128, 4, B], F32, tag="G1", name=f"G1{sfx}_{t}")
                G2 = pp.tile([128, 4, B], F32, tag="G2", name=f"G2{sfx}_{t}")
                for bank, G in ((0, G1), (1, G2)):
                    nc.tensor.matmul(
                        G[:].rearrange("p a b -> p (a b)"),
                        biasq[:, bank], qhot[:],
                        start=True, stop=False, skip_group_check=True)
                    for gc in range(4 * bank, 4 * bank + 4):
                        for kc in range(2):
                            nc.tensor.matmul(
                                G[:, gc % 4], wih[:, kc, gc], xt[:, kc, t],
                                start=False, stop=False, skip_group_check=True)
                return (G1, G2)

            # ---------------- LSTM chunk (roles 0 and 1) ----------------
            def emit_lstm_chunk(cj, is_l1, sfx):
                rk = 1 if is_l1 else 0
                xt = xt_pp[rk][cj % 2]
                if cj + 1 < NCH:
                    emit_xt_prefetch(cj + 1, is_l1)
                hTc = xp.tile([128, CH, 2, B], BF16, tag="hTc", name=f"hTc{sfx}")

                def emit_step(Gp, t):
                    G1, G2 = Gp
                    if t == 0:
                        prevT = hTp[:]
                    else:
                        prevT = hTc[:, t - 1]
                    for bank, G in ((0, G1), (1, G2)):
                        for gc in range(4 * bank, 4 * bank + 4):
                            for kc in range(2):
                                nc.tensor.matmul(
                                    G[:, gc % 4], whh[:, kc, gc], prevT[:, kc],
                                    start=False,
                                    stop=(gc % 4 == 3 and kc == 1),
                                    skip_group_check=True)
                    # gates: gc 0-1:i 2-3:f 4-5:g 6-7:o ; sg = [p, gate, half, b]
                    sg = sp.tile([128, 4, 2, B], BF16, tag="sg", name=f"sg{sfx}_{t}")
                    nc.scalar.activation(sg[:, 0:2], G1[:], AF.Sigmoid)
                    nc.scalar.activation(sg[:, 2], G2[:, 0:2], AF.Tanh)
                    nc.scalar.activation(sg[:, 3], G2[:, 2:4], AF.Sigmoid)
                    th = sp.tile([128, 2, B], BF16, tag="th", name=f"th{sfx}_{t}")
                    cf = sp.tile([128, 2, B], BF16, tag="cf", name=f"cf{sfx}_{t}")
                    nc.vector.tensor_mul(out=cf[:], in0=c_s[:], in1=sg[:, 1])
                    v = sp.tile([128, 2, B], BF16, tag="v", name=f"v{sfx}_{t}")
                    nc.vector.tensor_mul(out=v[:], in0=sg[:, 0], in1=sg[:, 2])
                    nc.vector.tensor_add(out=c_s[:], in0=cf[:], in1=v[:])
                    nc.scalar.activation(th[:], c_s[:], AF.Tanh)
                    nc.vector.tensor_mul(out=hTc[:, t], in0=sg[:, 3], in1=th[:])

                Gs = [emit_group(xt, 0, sfx), emit_group(xt, 1, sfx),
                      emit_group(xt, 2, sfx)]
                for t in range(CH):
                    if t + 3 < CH:
                        Gs.append(emit_group(xt, t + 3, sfx))
                    emit_step(Gs[t], t)

                # hand the last h^T to the next chunk via the persistent tile
                nc.vector.tensor_copy(out=hTp[:].rearrange("p k b -> p (k b)"),
                                      in_=hTc[:, CH - 1].rearrange(
                                          "p k b -> p (k b)"))
                # ship the chunk (per kc: DMA <= 3 dims)
                dst = agi_h1[cj] if is_l1 else agi_h0[cj]
                for kc in range(2):
                    nc.sync.dma_start(
                        out=dst.ap()[:, kc].rearrange("t p b -> p t b"),
                        in_=hTc[:, :, kc])

            # ---------------- helper: per-chunk a_pre + history ----------------
            def emit_helper_chunk(cj, sfx):
                ht = ht_pp[cj % 2]
                if cj + 1 < NCH:
                    for kc in range(2):
                        nc.sync.dma_start(
                            out=ht_pp[(cj + 1) % 2][:, kc],
                            in_=agb_h1[cj + 1].ap()[0:CH, kc]
                            .rearrange("t p b -> p t b"))
                # a_pre = Wa1_half^T @ h1 : [A, CH*B] in 8 passes of 4 t
                apo = ap_.tile([A, CH, B], BF16, tag="apo", name=f"apo{sfx}")
                for qu in range(8):
                    aps = pp.tile([A, 512], F32, tag="G1", name=f"aps{sfx}_{qu}")
                    t0 = qu * 4
                    for kc in range(2):
                        nc.tensor.matmul(
                            aps[:],
                            wa1[:, kc],
                            ht[:, kc, t0:t0 + 4].rearrange("p t b -> p (t b)"),
                            start=(kc == 0), stop=(kc == 1))
                    nc.vector.tensor_copy(
                        out=apo[:, t0:t0 + 4].rearrange("p t b -> p (t b)"),
                        in_=aps[:])
                nc.sync.dma_start(out=agi_ap[cj].ap(), in_=apo[:])

                # history: transpose h^T -> [B, CH, H] and stage to DRAM
                hbl = ap_.tile([B, CH, H], BF16, tag="hbl", name=f"hbl{sfx}")
                for t in range(CH):
                    TP1 = pp.tile([128, 8, B], BF16, tag="G1", name=f"tp1{sfx}_{t}")
                    TP2 = pp.tile([128, 8, B], BF16, tag="G2", name=f"tp2{sfx}_{t}")
                    nc.tensor.transpose(TP1[:, 0], ht[:, 0, t], identb[:])
                    nc.tensor.transpose(TP2[:, 0], ht[:, 1, t], identb[:])
                    nc.vector.tensor_copy(out=hbl[:, t, 0:128], in_=TP1[:, 0])
                    nc.vector.tensor_copy(out=hbl[:, t, 128:256], in_=TP2[:, 0])
                nc.sync.dma_start(out=hist_d.ap()[cj], in_=hbl[:])

            # ---------------- helper: window (score + weighted accum) ----------
            def emit_window(w, sfx):
                for bi in range(2):
                    cb = (MID - 1 - w) if bi == 0 else (MID + w)
                    apown = ap_.tile([A, CH, B], BF16, tag="apown",
                                     name=f"apw{sfx}_{bi}")
                    nc.sync.dma_start(out=apown[:], in_=agi_ap[cb].ap())
                    aprem = ap_.tile([A, CH, B], BF16, tag="aprem",
                                     name=f"apr{sfx}_{bi}")
                    nc.sync.dma_start(out=aprem[:], in_=apmir[w % 2].ap()[bi])
                    nc.vector.tensor_add(out=apown[:], in0=apown[:],
                                         in1=aprem[:][:, ::-1, :])
                    nc.scalar.activation(apown[:], apown[:], AF.Tanh)
                    # s[b, t] = sum_a wa2[a] * tanh[a, t, b]  (stationary = tanh_t)
                    spsum = pp.tile([128, 4, B], F32, tag="G1",
                                    name=f"sp{sfx}_{bi}")
                    for t in range(CH):
                        nc.tensor.matmul(spsum[:, 0, t:t + 1],
                                         apown[:, t], wa2[:],
                                         start=(t == 0), stop=(t == CH - 1))
                    sco = ap_.tile([128, CH], F32, tag="sco", name=f"sc{sfx}_{bi}")
                    nc.scalar.activation(sco[:], spsum[:, 0, 0:CH],
                                         AF.Sigmoid, bias=mneg[:, 0:1])
                    dtl = ap_.tile([128, CH], F32, tag="dtl", name=f"dt{sfx}_{bi}")
                    nc.vector.tensor_scalar(out=dtl[:], in0=sco[:],
                                            scalar1=-1.0, scalar2=1.0,
                                            op0=OP.mult, op1=OP.add)
                    nc.vector.reciprocal(out=dtl[:], in_=dtl[:])
                    e_blk = ap_.tile([128, CH], F32, tag="e_blk",
                                     name=f"eb{sfx}_{bi}")
                    dinc = ap_.tile([128, 1], F32, tag="dinc", name=f"di{sfx}_{bi}")
                    nc.vector.scalar_tensor_tensor(
                        out=e_blk[:], in0=sco[:], scalar=1.0, in1=dtl[:],
                        op0=OP.mult, op1=OP.mult, accum_out=dinc[:])
                    nc.vector.tensor_add(out=den[:], in0=den[:], in1=dinc[:])
                    hw_ = ap_.tile([B, CH, H], BF16, tag="hbl", name=f"hw{sfx}_{bi}")
                    nc.sync.dma_start(out=hw_[:], in_=hist_d.ap()[cb])
                    for u in range(CH):
                        nc.vector.scalar_tensor_tensor(
                            out=acc_v[:], in0=hw_[:, u], scalar=e_blk[:, u:u + 1],
                            in1=acc_v[:], op0=OP.mult, op1=OP.add)

            # ---------------- tick loop ----------------
            for tick in range(NT):
                if tick < NCH:
                    with tc.If(role == 0, name=f"L0t{tick}"):
                        emit_lstm_chunk(tick, False, f"a{tick}")
                    nc.gpsimd.collective_compute(
                        "AllGather", mybir.AluOpType.bypass,
                        replica_groups=G_H0,
                        ins=[agi_h0[tick].ap()], outs=[agb_h0[tick].ap()])

                if tick == LAG:
                    with tc.If(role == 1, name="L1pf"):
                        emit_xt_prefetch(0, True)

                j1 = tick - LAG
                if 0 <= j1 < NCH:
                    with tc.If(role == 1, name=f"L1t{tick}"):
                        emit_lstm_chunk(j1, True, f"b{tick}")
                    nc.gpsimd.collective_compute(
                        "AllGather", mybir.AluOpType.bypass,
                        replica_groups=G_H1,
                        ins=[agi_h1[j1].ap()], outs=[agb_h1[j1].ap()])

                j2 = tick - LAG - HLAG
                if j2 == -1:
                    with tc.If(role == 3, name="Hpf"):
                        for kc in range(2):
                            nc.sync.dma_start(
                                out=ht_pp[0][:, kc],
                                in_=agb_h1[0].ap()[0:CH, kc]
                                .rearrange("t p b -> p t b"))

                w = tick - WSTART
                # slot-asymmetric peer a_pre pulls into the apmir ring
                if 0 <= w < NW:
                    for slot, cpid in ((1, 5), (0, 7)):
                        with tc.If(pid == cpid, name=f"AR{tick}_{cpid}"):
                            for bi in range(2):
                                cb = (MID - 1 - w) if bi == 0 else (MID + w)
                                crem = NCH - 1 - cb
                                nc.sync.dma_start(
                                    out=apmir[w % 2].ap()[bi],
                                    in_=agb_ap[crem].ap()
                                    [slot * A:(slot + 1) * A])

                do_h = 0 <= j2 < NCH
                do_w = 0 <= w < NW
                if do_h or do_w or tick == NT - 1:
                    with tc.If(role == 3, name=f"Ht{tick}"):
                        if do_h:
                            emit_helper_chunk(j2, f"h{tick}")
                        if do_w:
                            emit_window(w, f"w{tick}")
                        if tick == NT - 1:
                            rden = ap_.tile([128, 1], F32, tag="dinc", name="rden")
                            nc.vector.reciprocal(out=rden[:], in_=den[:])
                            yt = ap_.tile([128, 256], F32, tag="yt", name="yt")
                            nc.vector.tensor_scalar_mul(yt[:], acc_v[:],
                                                        rden[:, 0:1])
                            nc.sync.dma_start(out=yout_d.ap(), in_=yt[:])

                if 0 <= j2 < NCH:
                    nc.gpsimd.collective_compute(
                        "AllGather", mybir.AluOpType.bypass,
                        replica_groups=G_AP,
                        ins=[agi_ap[j2].ap()], outs=[agb_ap[j2].ap()])

    nc.compile()
    return nc


def _prep_lstm_w(Wih, Whh, bih, bhh, bf16):
    # torch gate order i,f,g,o matches the device gc order directly.
    def tile_w(M):
        # [1024, K] -> [128(p=k%128), 2(kc), 8(gc), 128(c)]
        return np.ascontiguousarray(
            np.asarray(M, np.float32).T.reshape(2, 128, 8, 128)
            .transpose(1, 0, 2, 3)).astype(bf16)

    bias = (np.asarray(bih, np.float32) + np.asarray(bhh, np.float32))
    # biasq[r, bank, :] = bias[(4*bank + r)*128 : ...]
    biasq = np.ascontiguousarray(
        bias.reshape(2, 4, 128).transpose(1, 0, 2)).astype(bf16)
    return tile_w(Wih), tile_w(Whh), biasq


def kernel(**inputs):
    import ml_dtypes
    from concourse.bass_utils import run_bass_kernel_spmd

    bf16 = ml_dtypes.bfloat16
    x = np.asarray(inputs["x"], np.float32)
    Bv, S, Dv = x.shape
    if (S, "nc") not in _BUILD_CACHE:
        _BUILD_CACHE[(S, "nc")] = _build(S)
    nc = _BUILD_CACHE[(S, "nc")]

    xs_f = np.ascontiguousarray(x.transpose(1, 2, 0)).reshape(
        S, 2, 128, Bv).astype(bf16)
    xs_b = np.ascontiguousarray(x[:, ::-1].transpose(1, 2, 0)).reshape(
        S, 2, 128, Bv).astype(bf16)
    z_xs = np.zeros((S, 2, 128, Bv), bf16)
    zw = np.zeros((128, 2, 8, 128), bf16)
    zb = np.zeros((4, 2, 128), bf16)

    wf0 = _prep_lstm_w(inputs["Wih_f0"], inputs["Whh_f0"], inputs["bih_f0"],
                       inputs["bhh_f0"], bf16)
    wf1 = _prep_lstm_w(inputs["Wih_f1"], inputs["Whh_f1"], inputs["bih_f1"],
                       inputs["bhh_f1"], bf16)
    wb0 = _prep_lstm_w(inputs["Wih_b0"], inputs["Whh_b0"], inputs["bih_b0"],
                       inputs["bhh_b0"], bf16)
    wb1 = _prep_lstm_w(inputs["Wih_b1"], inputs["Whh_b1"], inputs["bih_b1"],
                       inputs["bhh_b1"], bf16)

    qhot = np.zeros((4, 512), np.float32)
    for r in range(4):
        qhot[r, r * 128:(r + 1) * 128] = 1.0
    qhot = qhot.astype(bf16)

    wa1 = np.asarray(inputs["Wa1"], np.float32)          # [A, 2H]
    wa2 = np.asarray(inputs["Wa2"], np.float32).reshape(A)

    def wa1half(cols):
        # [A, 256] -> [128(p), 2(kc), A]
        return np.ascontiguousarray(
            cols.T.reshape(2, 128, A).transpose(1, 0, 2)).astype(bf16)

    wa1f = wa1half(wa1[:, 0:H])
    wa1b = wa1half(wa1[:, H:2 * H])
    zwa1 = np.zeros((128, 2, A), bf16)
    wa2c = wa2.reshape(A, 1).astype(bf16)
    mconst = np.float32(np.abs(wa2).sum())
    mneg = np.full((1, 1), -mconst, np.float32)

    def imap(xs, w3, wa1t):
        wih, whh, biasq = w3
        return {"xs": xs, "wihT": wih, "whhT": whh, "biasq": biasq,
                "qhot": qhot, "wa1T": wa1t, "wa2c": wa2c, "mneg": mneg}

    zero3 = (zw, zw, zb)
    in_maps = [
        imap(xs_f, wf0, zwa1), imap(z_xs, wf1, zwa1),
        imap(xs_b, wb0, zwa1), imap(z_xs, wb1, zwa1),
        imap(z_xs, zero3, zwa1), imap(z_xs, zero3, wa1f),
        imap(z_xs, zero3, zwa1), imap(z_xs, zero3, wa1b),
    ]
    global _last_in_maps
    _last_in_maps = in_maps
    res = run_bass_kernel_spmd(nc, in_maps, core_ids=list(range(8)))
    out = np.concatenate([res.results[5]["yout"], res.results[7]["yout"]], axis=1)
    return out.astype(np.float32)
